# revision 32
# baseline (speedup 1.0000x reference)
"""Trainium2 Bass kernel for nn_LDS_LR: low-rank LDS + AR low-rank correction.

Math (per batch b):
    Bu   = X @ B1 @ B2                      # [T, N] rank-64 input projection
    h_t  = A * h_{t-1} + Bu_t               # diagonal recurrence, h_{-1} = h0
    lds  = H @ C1 @ C2                      # [T, O] rank-64 output projection
    proj = einsum('ti,rik->trk', X, M1)     # [T, R, KX]
    ar_t = sum_k M2[:,:,k] @ proj[t-k,:,k]  # AR with KX=5 taps
    Y    = lds + ar

Sharding: 8 cores = 4 batches x 2 sequence halves (1024 steps each).

v4 design notes (on top of v3's sorted-A windowed carries):
  * States host-permuted by |A| asc; windowed decay matmuls for the carry V
    and the CH1 corrections (~2k cols each instead of 8k).
  * Scan-then-correct at BOTH levels: block-1 scans start from zero, the
    missed A^(t'+1)*h_511 term is folded into CH1(1) as one more windowed
    matmul reusing the same apc slices.  All 16 scans are then independent:
    no serial 20us DVE chain; a few scans can offload to the Pool engine.
  * The carry-only inputs (xp, B1 copy, apv) travel as fp8e4m3 — the carry
    is a small additive term so 6% quantization there is ~0.3% on Y.  B1 is
    pre-scaled x1024 into fp8 range; the ones-reduction column carries the
    1/1024 compensation.
  * G matmul paired into PE column groups (rows 0:64 = tb0, 64:128 = tb1):
    half the column count and one eviction instead of two.
  * PE warm-up matmuls on a memset tile from t~0 keep the HAM clock-gate at
    2.4 GHz; input DMAs spread over the 3 hardware queues, k-loops consume
    in arrival order; xp scheduled mid-stream (carry chain needs it late).
"""

import contextlib
import ctypes
import os
import sys
import types

import numpy as np
from contextlib import ExitStack

import concourse.bass as bass
import concourse.tile as tile
from concourse import bacc, mybir
from concourse.bass_utils import run_bass_kernel_spmd


def _install_ntff_hook():
    """Provide antenv.axon_hooks.get_axon_ntff_profile_hook if the image
    lacks it, driving NTFF capture via the libaxon_pjrt C ABI directly."""
    try:
        from antenv.axon_hooks import get_axon_ntff_profile_hook  # noqa: F401
        return
    except ImportError:
        pass
    so_path = "/opt/axon/libaxon_pjrt.so"
    hook = None
    if os.path.exists(so_path):
        lib = ctypes.CDLL(so_path)
        if hasattr(lib, "axon_start_nrt_profile"):
            lib.axon_start_nrt_profile.argtypes = [
                ctypes.POINTER(ctypes.c_int64), ctypes.c_size_t]
            lib.axon_start_nrt_profile.restype = ctypes.c_int64
            lib.axon_stop_nrt_profile.argtypes = [ctypes.c_char_p]
            lib.axon_stop_nrt_profile.restype = ctypes.c_int64

            @contextlib.contextmanager
            def _hook(output_dir, device_ids):
                import jax
                jax.devices()
                if device_ids:
                    ids = (ctypes.c_int64 * len(device_ids))(*device_ids)
                    rc = lib.axon_start_nrt_profile(ids, len(device_ids))
                else:
                    rc = lib.axon_start_nrt_profile(None, 0)
                if rc != 0:
                    raise RuntimeError(f"axon_start_nrt_profile rc={rc}")
                try:
                    yield
                finally:
                    n = lib.axon_stop_nrt_profile(str(output_dir).encode())
                    print(f"ntff profile: {n} file(s) -> {output_dir}",
                          file=sys.stderr)

            hook = _hook
    mod = types.ModuleType("antenv.axon_hooks")
    mod.get_axon_ntff_profile_hook = lambda: hook
    mod.set_axon_ntff_profile_hook = lambda h: None
    sys.modules["antenv.axon_hooks"] = mod


_install_ntff_hook()

DT = mybir.dt.float32
MDT = mybir.dt.bfloat16
F8 = mybir.dt.float8e4
MNP = mybir.dt.np(MDT)
F8NP = mybir.dt.np(F8)
F32 = np.float32
ODT = MDT
ONP = mybir.dt.np(ODT)

B, T, D = 4, 2048, 1024
NST, R, KX, OUT = 1024, 64, 5, 1024
TC = 1024          # per-core chunk length
TBL = 512          # time block (one PSUM bank at fp32)

# decay windows per sorted n-tile (compile-time; states sorted by A asc)
APW = 2048         # apv/apc width: 1024 + 256 + 6*128
B2W = 1152         # b2x width: 1024 B2 + 64 ident + ones + pad

WARM_MM = int(os.environ.get("KERNEL_WARM_MM", "10"))
# scans whose Bu is staged PSUM->SBUF (ACT copy) so the DVE scan runs in the
# all-SBUF 2x mode; remaining scans read PSUM directly at 1x
STAGE_SCAN = int(os.environ.get("KERNEL_STAGE_SCAN", "0"))
USE_F8 = bool(int(os.environ.get("KERNEL_F8", "1")))      # fp8 carry path
# col-group-paired G matmul: faults trn2 hardware when the operands are fp8
# (bf16 pairing and unpaired fp8 both pass) — keep off
PAIR_G = bool(int(os.environ.get("KERNEL_PAIR_G", "0")))
F8D = F8 if USE_F8 else MDT   # dtype of the carry-path tensors
F8DNP = mybir.dt.np(F8D)

_CACHED_NC = None
LAST_RESULT = None  # BassKernelResults of the most recent run (for test.py)

MULT = mybir.AluOpType.mult
ADD = mybir.AluOpType.add

# xo k-tile consumption order ~ DMA arrival (q0,q2 sync / q1 gpsimd /
# q3 scalar-after-w1g)
KORD = [0, 4, 1, 5, 2, 6, 3, 7]


def _emit(ctx, tc, io):
    nc = tc.nc
    xo, xp, xtl, w1, w1g8, b2x, c1, w2, apv, apc, avio, yt = io

    wp = ctx.enter_context(tc.tile_pool(name="wp", bufs=1))
    xpool = ctx.enter_context(tc.tile_pool(name="xpool", bufs=1))
    hp = ctx.enter_context(tc.tile_pool(name="hp", bufs=1))
    pp = ctx.enter_context(tc.tile_pool(name="pp", bufs=1))
    yp = ctx.enter_context(tc.tile_pool(name="yp", bufs=1))
    pA = ctx.enter_context(tc.tile_pool(name="pA", bufs=3, space="PSUM"))
    pB = ctx.enter_context(tc.tile_pool(name="pB", bufs=3, space="PSUM"))
    pC = ctx.enter_context(tc.tile_pool(name="pC", bufs=2, space="PSUM"))

    # ---------------- warm-up: memset tile + dummy accumulating MMs ---------
    wtile = wp.tile([128, 640], MDT, tag="wtile", name="wtile")
    nc.gpsimd.memset(wtile[:], 0.0)
    wps = pC.tile([128, TBL], DT, tag="pc", name="wps")
    for i in range(WARM_MM):
        nc.tensor.matmul(wps[:], wtile[:, 512:640], wtile[:, 0:512],
                         start=(i == 0), stop=(i == WARM_MM - 1))

    # ---------------- input DMAs, 3 queues, deadline-ordered ----------------
    aviosb = wp.tile([128, 16], DT, tag="avio", name="aviosb")
    xots = [xpool.tile([128, 2048], MDT, tag=f"xoq{i}", name=f"xoq{i}")
            for i in range(4)]
    xpts = [xpool.tile([128, 2048], F8D, tag=f"xpq{i}", name=f"xpq{i}")
            for i in range(4)]
    c1sb = wp.tile([128, 512], MDT, tag="c1", name="c1sb")
    w1gsb = wp.tile([128, 1024], MDT, tag="w1g", name="w1gsb")
    w1rsb = wp.tile([128, 2048], MDT, tag="w1r", name="w1rsb")
    w1g8sb = wp.tile([128, 512], F8D, tag="w1g8", name="w1g8sb")
    xtlsb = wp.tile([128, 32], MDT, tag="xtl", name="xtlsb")
    b2xsb = wp.tile([128, B2W], MDT, tag="b2x", name="b2xsb")
    w2sb = wp.tile([128, 3072], MDT, tag="w2", name="w2sb")
    apvsb = wp.tile([128, APW], F8D, tag="apv", name="apvsb")
    apcsb = wp.tile([128, APW], MDT, tag="apc", name="apcsb")

    # sync queue: xo first, then xp quarter + corr weights
    nc.sync.dma_start(aviosb[:], avio[:])
    nc.sync.dma_start(xots[0][:], xo[:, 0:2048])
    nc.sync.dma_start(xots[1][:], xo[:, 2048:4096])
    nc.sync.dma_start(xpts[0][:], xp[:, 0:2048])
    nc.sync.dma_start(c1sb[:], c1[:])
    nc.sync.dma_start(apcsb[:], apc[:])
    # gpsimd queue: xo second half, then xp
    nc.gpsimd.dma_start(xots[2][:], xo[:, 4096:6144])
    nc.gpsimd.dma_start(xots[3][:], xo[:, 6144:8192])
    nc.gpsimd.dma_start(xpts[1][:], xp[:, 2048:4096])
    nc.gpsimd.dma_start(xpts[2][:], xp[:, 4096:6144])
    nc.gpsimd.dma_start(xpts[3][:], xp[:, 6144:8192])
    # scalar queue: all weights
    nc.scalar.dma_start(w1gsb[:], w1[:, 0:1024])
    nc.scalar.dma_start(w1g8sb[:], w1g8[:])
    nc.scalar.dma_start(b2xsb[:], b2x[:])
    nc.scalar.dma_start(w1rsb[:], w1[:, 1024:3072])
    nc.scalar.dma_start(xtlsb[:], xtl[:])
    nc.scalar.dma_start(apvsb[:], apv[:])
    nc.scalar.dma_start(w2sb[:], w2[:])

    def xot(k):
        return xots[k // 2][:, (k % 2) * 1024:(k % 2 + 1) * 1024]

    def xpt(k):
        return xpts[k // 2][:, (k % 2) * 1024:(k % 2 + 1) * 1024]

    def w1t(k, lo, hi):  # W1 k-tile column slice (w1g: cols 0:128, w1r: rest)
        if hi <= 128:
            return w1gsb[:, k * 128 + lo:k * 128 + hi]
        assert lo >= 128
        return w1rsb[:, k * 256 + lo - 128:k * 256 + hi - 128]

    def w2t(m, o):  # W2 stationary for (m-tile, o-tile) [128, 128]
        return w2sb[:, m * 1024 + o * 128:m * 1024 + (o + 1) * 128]

    def abv(n):  # A broadcast for scans, stride-0 partition view
        return aviosb[:, n:n + 1].broadcast_to((128, TBL))

    # ---------------- j0 = [B1|tap0]^T Xo -----------------------------------
    PW = 4 + TC + 4
    pext = [pp.tile([128, PW], MDT, tag=f"pext{j}", name=f"pext{j}")
            for j in range(3)]
    j0ps = [pA.tile([128, TBL], DT, tag="pa", name=f"j0_ps{t}")
            for t in range(2)]
    for i, k in enumerate(KORD):
        for t in range(2):
            nc.tensor.matmul(j0ps[t][:], w1t(k, 0, 128),
                             xot(k)[:, t * TBL:(t + 1) * TBL],
                             start=(i == 0), stop=(i == 7))
    for t in range(2):
        nc.scalar.copy(pext[0][:, 4 + t * TBL:4 + (t + 1) * TBL], j0ps[t][:])

    # ---------------- Bu + 16 independent scans -----------------------------
    # b0 scans start from the host-folded h0 offset; b1 scans start from 0 and
    # the missed A^(t'+1)*h_511 term lands in CH1(1) via the sc2 correction.
    hsb = [hp.tile([128, TC], MDT, tag=f"h{n}", name=f"h{n}") for n in range(8)]

    def emit_buo(n, tb):
        bu = pB.tile([128, TBL], DT, tag="pb", name=f"buo{n}_{tb}")
        nc.tensor.matmul(bu[:], b2xsb[0:64, n * 128:(n + 1) * 128],
                         pext[0][0:64, 4 + tb * TBL:4 + (tb + 1) * TBL],
                         start=True, stop=True)
        init = aviosb[:, 8 + n:9 + n] if tb == 0 else 0.0
        dst = hsb[n][:, tb * TBL:(tb + 1) * TBL]
        nc.vector.tensor_tensor_scan(dst, abv(n), bu[:], init, MULT, ADD)

    emit_buo(0, 0)
    emit_buo(1, 0)

    # ---------------- carry chain: G = B1^T Xp, G^T, V, E, D ----------------
    gsb = wp.tile([64, 1024], MDT, tag="gprev", name="gsb")
    g_ps = [pC.tile([64, TBL], DT, tag="pc", name=f"g_ps{t}")
            for t in range(2)]
    for i, k in enumerate(range(8)):
        st = w1g8sb[:, k * 64:(k + 1) * 64]
        for t in range(2):
            nc.tensor.matmul(g_ps[t][:], st,
                             xpt(k)[:, t * TBL:(t + 1) * TBL],
                             start=(i == 0), stop=(i == 7))
    for t in range(2):
        nc.scalar.copy(gsb[:, t * TBL:(t + 1) * TBL], g_ps[t][:])

    emit_buo(2, 0)

    # gt[:, st*64:(st+1)*64] = (G[:, st*128:(st+1)*128])^T via identity MMs
    gtsb = wp.tile([128, 512], F8D, tag="gt", name="gtsb")
    gt_ps = pC.tile([128, 512], DT, tag="pc", name="gt_ps")
    for st in range(8):
        nc.tensor.matmul(gt_ps[:, st * 64:(st + 1) * 64],
                         gsb[:, st * 128:(st + 1) * 128],
                         b2xsb[0:64, 1024:1088], start=True, stop=True)
    nc.scalar.copy(gtsb[:], gt_ps[:])

    emit_buo(3, 0)
    emit_buo(4, 0)

    # ---------------- j1 / j2 (with xtl-fed boundary tails) -----------------
    def emit_j(j, klo, khi, jps, tl):
        for i in range(klo, khi):
            k = KORD[i]
            st = w1t(k, j * 128, (j + 1) * 128)
            for t in range(2):
                nc.tensor.matmul(jps[t][:], st,
                                 xot(k)[:, t * TBL:(t + 1) * TBL],
                                 start=(i == 0), stop=(i == 7))
            nc.tensor.matmul(tl[:], st, xtlsb[:, k * 4:(k + 1) * 4],
                             start=(i == 0), stop=(i == 7))

    def evict_j(j, jps, tl):
        ka, kb = 2 * j - 1, 2 * j
        for t in range(2):
            nc.scalar.copy(
                pext[j][0:64, 4 + ka + t * TBL:4 + ka + (t + 1) * TBL],
                jps[t][0:64, :])
            nc.scalar.copy(
                pext[j][64:128, 4 + kb + t * TBL:4 + kb + (t + 1) * TBL],
                jps[t][64:128, :])
        nc.scalar.copy(pext[j][0:64, 4:4 + ka], tl[0:64, 4 - ka:4])
        nc.scalar.copy(pext[j][64:128, 4:4 + kb], tl[64:128, 4 - kb:4])

    j1ps = [pC.tile([128, TBL], DT, tag="pc", name=f"j1_ps{t}")
            for t in range(2)]
    tl1 = pA.tile([128, 4], DT, tag="pa", name="tl1")
    emit_j(1, 0, 4, j1ps, tl1)
    emit_buo(5, 0)

    # V[r, n] = sum_s G[r, s] A[n]^(1023-s), windowed by sorted-A tiles.
    # apv segments: [0:1024]=st7 all n; [1024:1280]=st6 n 768:1024;
    # [1280+st*128 ...]=st 0..5, n 896:1024.
    v_ps = [pC.tile([64, TBL], DT, tag="pc", name=f"v_ps{nh}")
            for nh in range(2)]
    nc.tensor.matmul(v_ps[0][:], gtsb[:, 448:512], apvsb[:, 0:512],
                     start=True, stop=True)
    nc.tensor.matmul(v_ps[1][:], gtsb[:, 448:512], apvsb[:, 512:1024],
                     start=True, stop=False)
    nc.tensor.matmul(v_ps[1][:, 256:512], gtsb[:, 384:448],
                     apvsb[:, 1024:1280], start=False, stop=False)
    for st in range(6):
        nc.tensor.matmul(v_ps[1][:, 384:512], gtsb[:, st * 64:(st + 1) * 64],
                         apvsb[:, 1280 + st * 128:1280 + (st + 1) * 128],
                         start=False, stop=(st == 5))

    # E = V * B2 elementwise (DVE — slots between scans 5 and 6);
    # D[n] = sum_r E[r, n] * (1/1024 ones-matmul)
    esb = wp.tile([64, 1024], MDT, tag="esb", name="esb")
    for nh in range(2):
        nc.vector.scalar_tensor_tensor(
            esb[:, nh * TBL:(nh + 1) * TBL], v_ps[nh][:], 1.0,
            b2xsb[0:64, nh * TBL:(nh + 1) * TBL], MULT, MULT)
    emit_buo(6, 0)
    emit_j(1, 4, 8, j1ps, tl1)
    evict_j(1, j1ps, tl1)
    emit_buo(7, 0)

    j2ps = [pC.tile([128, TBL], DT, tag="pc", name=f"j2_ps{t}")
            for t in range(2)]
    tl2 = pA.tile([128, 4], DT, tag="pa", name="tl2")
    emit_j(2, 0, 4, j2ps, tl2)
    emit_buo(0, 1)
    emit_j(2, 4, 8, j2ps, tl2)
    evict_j(2, j2ps, tl2)
    d_ps = pA.tile([128, 8], DT, tag="pa", name="d_ps")
    for n in range(8):
        nc.tensor.matmul(d_ps[:, n:n + 1], esb[:, n * 128:(n + 1) * 128],
                         b2xsb[0:64, 1088:1089], start=True, stop=True)
    emit_buo(1, 1)

    # correction stationaries on the ACT engine: scorr = C1*D (chunk carry),
    # sc2 = C1*h_511 (block-1 zero-init carry).  Emitted after ALL b0 scans
    # so the hsb reads depend on the scan writes.
    scorr = wp.tile([128, 512], MDT, tag="scorr", name="scorr")
    sc2 = wp.tile([128, 512], MDT, tag="sc2", name="sc2")
    h511f = wp.tile([128, 8], DT, tag="h511f", name="h511f")
    dsb = wp.tile([128, 8], DT, tag="dsb", name="dsb")
    nc.scalar.copy(dsb[:], d_ps[:])
    for nt in range(8):
        nc.scalar.mul(scorr[:, nt * 64:(nt + 1) * 64],
                      c1sb[:, nt * 64:(nt + 1) * 64], dsb[:, nt:nt + 1])
    for nt in range(8):
        nc.scalar.copy(h511f[:, nt:nt + 1], hsb[nt][:, TBL - 1:TBL])
        nc.scalar.mul(sc2[:, nt * 64:(nt + 1) * 64],
                      c1sb[:, nt * 64:(nt + 1) * 64], h511f[:, nt:nt + 1])

    # ---------------- CH1(tb) + windowed corrections, Y(tb) -----------------
    ysb = [[yp.tile([128, 4 * TBL], ODT, tag=f"y{tb}{g}", name=f"y{tb}{g}")
            for g in range(2)] for tb in range(2)]

    def corr_mms(cps, stat, last_stop):
        # windowed A^(t+1) correction: tile7 full 512, tile6 256, rest 128
        nc.tensor.matmul(cps[:], stat[:, 448:512], apcsb[:, 0:512],
                         start=False, stop=False)
        nc.tensor.matmul(cps[:, 0:256], stat[:, 384:448],
                         apcsb[:, 1024:1280], start=False, stop=False)
        for nt in range(6):
            nc.tensor.matmul(
                cps[:, 0:128], stat[:, nt * 64:(nt + 1) * 64],
                apcsb[:, 1280 + nt * 128:1280 + (nt + 1) * 128],
                start=False, stop=(last_stop and nt == 5))

    def emit_ch1(tb):
        cps = pC.tile([64, TBL], DT, tag="pc", name=f"c_ps{tb}")
        for n in range(8):
            nc.tensor.matmul(cps[:], c1sb[:, n * 64:(n + 1) * 64],
                             hsb[n][:, tb * TBL:(tb + 1) * TBL],
                             start=(n == 0), stop=False)
        if tb == 0:
            corr_mms(cps, scorr, True)
        else:
            # chunk carry at t 512:1024 decays below cutoff except tile 7
            nc.tensor.matmul(cps[:], scorr[:, 448:512], apcsb[:, 512:1024],
                             start=False, stop=False)
            corr_mms(cps, sc2, True)
        nc.scalar.copy(pext[0][0:64, 4 + tb * TBL:4 + (tb + 1) * TBL], cps[:])

    # Y: per o-tile accumulate m=1, m=2 early; the CH1-dependent m=0 last.
    yq = {}

    def y_mm12(tb, o):
        pool, tag = (pA, "pa") if tb == 0 else (pB, "pb")
        yps = pool.tile([128, TBL], DT, tag=tag, name=f"y_ps{o}_{tb}")
        yq[(tb, o)] = yps
        nc.tensor.matmul(yps[:], w2t(1, o),
                         pext[1][:, 4 + tb * TBL:4 + (tb + 1) * TBL],
                         start=True, stop=False)
        nc.tensor.matmul(yps[:], w2t(2, o),
                         pext[2][:, 4 + tb * TBL:4 + (tb + 1) * TBL],
                         start=False, stop=False)

    def y_mm0(tb, o):
        yps = yq.pop((tb, o))
        nc.tensor.matmul(yps[:], w2t(0, o),
                         pext[0][:, 4 + tb * TBL:4 + (tb + 1) * TBL],
                         start=False, stop=True)
        g, oo = divmod(o, 4)
        dst = ysb[tb][g][:, oo * TBL:(oo + 1) * TBL]
        if (tb == 0 and o < 4) or (tb == 1 and o % 2 == 1):
            nc.scalar.copy(dst, yps[:])
        else:
            nc.vector.tensor_copy(dst, yps[:])
        if oo == 3:
            eng = [nc.sync, nc.gpsimd, nc.gpsimd, nc.sync][tb * 2 + g]
            eng.dma_start(
                yt[:, tb * 4096 + g * 2048:tb * 4096 + (g + 1) * 2048],
                ysb[tb][g][:])

    emit_ch1(0)
    emit_buo(2, 1)

    # CH1(1): n-matmuls interleaved into the Y(0) pipeline (each waits only
    # its own b1 scan); the s(7,1)-dependent pieces + corrections come last.
    cps1 = pC.tile([64, TBL], DT, tag="pc", name="c_ps1")

    def ch1_1_n(n, start=False):
        nc.tensor.matmul(cps1[:], c1sb[:, n * 64:(n + 1) * 64],
                         hsb[n][:, TBL:TC], start=start, stop=False)

    ch1_1_n(0, start=True)
    y_mm12(0, 0)
    y_mm12(0, 1)
    y_mm12(0, 2)
    y_mm0(0, 0)
    emit_buo(3, 1)
    ch1_1_n(1)
    y_mm12(0, 3)
    y_mm0(0, 1)
    y_mm12(0, 4)
    y_mm0(0, 2)
    emit_buo(4, 1)
    ch1_1_n(2)
    y_mm12(0, 5)
    y_mm0(0, 3)
    y_mm12(0, 6)
    y_mm0(0, 4)
    emit_buo(5, 1)
    ch1_1_n(3)
    y_mm12(0, 7)
    y_mm0(0, 5)
    y_mm0(0, 6)
    emit_buo(6, 1)
    ch1_1_n(4)
    y_mm0(0, 7)
    y_mm12(1, 0)
    emit_buo(7, 1)
    ch1_1_n(5)
    y_mm12(1, 1)
    y_mm12(1, 2)
    ch1_1_n(6)
    ch1_1_n(7)
    nc.tensor.matmul(cps1[:], scorr[:, 448:512], apcsb[:, 512:1024],
                     start=False, stop=False)
    corr_mms(cps1, sc2, True)
    nc.vector.tensor_copy(pext[0][0:64, 4 + TBL:4 + TC], cps1[:])
    y_mm0(1, 0)
    y_mm12(1, 3)
    y_mm0(1, 1)
    y_mm12(1, 4)
    y_mm0(1, 2)
    y_mm12(1, 5)
    y_mm0(1, 3)
    y_mm12(1, 6)
    y_mm0(1, 4)
    y_mm12(1, 7)
    y_mm0(1, 5)
    y_mm0(1, 6)
    y_mm0(1, 7)


def _build():
    nc = bacc.Bacc("TRN2", target_bir_lowering=False, debug=False,
                   num_devices=8)
    xo = nc.dram_tensor("xo", [128, 8192], MDT, kind="ExternalInput").ap()
    xp = nc.dram_tensor("xp", [128, 8192], F8D, kind="ExternalInput").ap()
    xtl = nc.dram_tensor("xtl", [128, 32], MDT, kind="ExternalInput").ap()
    w1 = nc.dram_tensor("w1", [128, 3072], MDT, kind="ExternalInput").ap()
    w1g8 = nc.dram_tensor("w1g8", [128, 512], F8D, kind="ExternalInput").ap()
    b2x = nc.dram_tensor("b2x", [128, B2W], MDT, kind="ExternalInput").ap()
    c1 = nc.dram_tensor("c1", [128, 512], MDT, kind="ExternalInput").ap()
    w2 = nc.dram_tensor("w2", [128, 3072], MDT, kind="ExternalInput").ap()
    apv = nc.dram_tensor("apv", [128, APW], F8D, kind="ExternalInput").ap()
    apc = nc.dram_tensor("apc", [128, APW], MDT, kind="ExternalInput").ap()
    avio = nc.dram_tensor("avio", [128, 16], DT, kind="ExternalInput").ap()
    yt = nc.dram_tensor("yt", [128, 8192], ODT, kind="ExternalOutput").ap()

    with tile.TileContext(nc) as tc, ExitStack() as ctx:
        _emit(ctx, tc, (xo, xp, xtl, w1, w1g8, b2x, c1, w2, apv, apc,
                        avio, yt))
    nc.compile()
    return nc


def _get_nc():
    global _CACHED_NC
    if _CACHED_NC is None:
        _CACHED_NC = _build()
    return _CACHED_NC


def _pack_kt(arr):
    """[1024, C] -> [128, 8*C] with blocks of 128 rows side by side."""
    C = arr.shape[1]
    return np.ascontiguousarray(
        arr.reshape(8, 128, C).transpose(1, 0, 2).reshape(128, 8 * C))


def kernel(inputs, h0, A, B1, B2, C1, C2, M1, M2):
    global LAST_RESULT
    X = np.asarray(inputs, dtype=F32)
    h0 = np.asarray(h0, dtype=F32)
    A = np.asarray(A, dtype=F32)

    # sort states by A ascending (weights-only permutation)
    perm = np.argsort(np.asarray(A, dtype=np.float64))
    As64 = np.asarray(A, dtype=np.float64)[perm]
    h0s = h0[perm]
    B2s = np.asarray(B2, dtype=F32)[:, perm]
    C1s = np.asarray(C1, dtype=F32)[perm, :]

    # sanity: windows hold for this A draw (program structure is fixed)
    assert As64[767] ** 128 < 1e-4, As64[767]
    assert As64[895] ** 256 < 1e-4, As64[895]

    W1 = np.concatenate(
        [np.asarray(B1, dtype=F32)]
        + [np.ascontiguousarray(np.asarray(M1, dtype=F32)[:, :, k].T)
           for k in range(KX)], axis=1)
    W2 = np.concatenate(
        [np.asarray(C2, dtype=F32)]
        + [np.ascontiguousarray(np.asarray(M2, dtype=F32)[:, :, k].T)
           for k in range(KX)], axis=0)
    w1kt = W1.astype(MNP).reshape(8, 128, 384)           # [k, p, c]
    w1p = np.concatenate([                               # [128, 1024 | 2048]
        np.ascontiguousarray(w1kt[:, :, 0:128].transpose(1, 0, 2)
                             .reshape(128, 1024)),
        np.ascontiguousarray(w1kt[:, :, 128:384].transpose(1, 0, 2)
                             .reshape(128, 2048))], axis=1)
    # fp8 B1 copy, pre-scaled x1024 into e4m3 range (G path only)
    b1s = (np.asarray(B1, dtype=F32) * 1024.0).astype(F8DNP)  # [1024, 64]
    w1g8p = np.ascontiguousarray(
        b1s.reshape(8, 128, 64).transpose(1, 0, 2).reshape(128, 512))
    w2p = np.ascontiguousarray(
        W2.reshape(3, 128, 1024).transpose(1, 0, 2)
        .reshape(128, 3072).astype(MNP))                 # [128, 3072]
    b2xm = np.zeros((128, B2W), F32)
    b2xm[0:64, 0:1024] = B2s
    b2xm[0:64, 1024:1088] = np.eye(64, dtype=F32)
    b2xm[64:128, 1024:1088] = np.eye(64, dtype=F32)
    b2xm[0:64, 1088] = 1.0 / 1024.0
    b2xp = np.ascontiguousarray(b2xm.astype(MNP))
    c1p = _pack_kt(C1s.astype(MNP))                      # [128, 512]

    # apv: V decay powers A^(1023-s), windowed; s = st*128 + p.
    lnAs = np.log(As64)
    p = np.arange(128, dtype=np.float64)
    apvm = np.zeros((128, APW), np.float64)
    apvm[:, 0:1024] = np.exp(np.outer(127.0 - p, lnAs))              # st7
    apvm[:, 1024:1280] = np.exp(np.outer(255.0 - p, lnAs[768:1024]))  # st6
    for st in range(6):
        apvm[:, 1280 + st * 128:1280 + (st + 1) * 128] = np.exp(
            np.outer(1023.0 - (st * 128 + p), lnAs[896:1024]))
    apvp = np.ascontiguousarray(apvm.astype(F32).astype(F8DNP))

    # apc: correction decay powers A^(t+1), windowed per sorted n-tile.
    t1 = np.arange(1, 1025, dtype=np.float64)
    apcm = np.zeros((128, APW), np.float64)
    apcm[:, 0:1024] = np.exp(np.outer(lnAs[896:1024], t1))           # tile7
    apcm[:, 1024:1280] = np.exp(np.outer(lnAs[768:896], t1[0:256]))  # tile6
    for nt in range(6):
        apcm[:, 1280 + nt * 128:1280 + (nt + 1) * 128] = np.exp(
            np.outer(lnAs[nt * 128:(nt + 1) * 128], t1[0:128]))
    apcp = np.ascontiguousarray(apcm.astype(F32).astype(MNP))

    ioff_h0 = h0s.astype(F32)                              # half 0: plain h0
    ioff_h1 = (As64 ** TC * h0s.astype(np.float64)).astype(F32)  # A^1024 h0

    Xbf = X.astype(MNP)
    X8 = X.astype(F8DNP)
    zeros_xp = np.zeros((128, 8192), F8DNP)
    zeros_xtl = np.zeros((128, 32), MNP)

    def pack_x(xarr, b, sl):
        return _pack_kt(np.ascontiguousarray(xarr[b, sl, :].T))

    in_maps = []
    for c in range(8):
        b, half = divmod(c, 2)
        xoc = pack_x(Xbf, b, slice(half * TC, (half + 1) * TC))
        if half == 0:
            xpc, xtlc, ioff = zeros_xp, zeros_xtl, ioff_h0
        else:
            xpc = pack_x(X8, b, slice(0, TC))
            xpbf = pack_x(Xbf, b, slice(0, TC))
            # xtl: last 4 time-cols of each k-tile of xp, [128, 8*4]
            xtlc = np.ascontiguousarray(
                xpbf.reshape(128, 8, 1024)[:, :, 1020:1024].reshape(128, 32))
            ioff = ioff_h1
        aviom = np.zeros((128, 16), F32)
        aviom[:, 0:8] = As64.astype(F32).reshape(8, 128).T
        aviom[:, 8:16] = ioff.reshape(8, 128).T
        in_maps.append({"xo": xoc, "xp": xpc, "xtl": xtlc, "w1": w1p,
                        "w1g8": w1g8p, "b2x": b2xp, "c1": c1p, "w2": w2p,
                        "apv": apvp, "apc": apcp, "avio": aviom})

    nc = _get_nc()
    trace = bool(int(os.environ.get("KERNEL_TRACE", "0")))
    LAST_RESULT = run_bass_kernel_spmd(nc, in_maps, core_ids=list(range(8)),
                                       trace=trace)
    Y = np.empty((B, T, OUT), F32)
    for c in range(8):
        b, half = divmod(c, 2)
        ytc = np.asarray(LAST_RESULT.results[c]["yt"], dtype=F32)
        # yt[p, tb*4096 + o*512 + t] -> Y_core[o*128+p, tb*512+t]
        yc = ytc.reshape(128, 2, 8, 512).transpose(2, 0, 1, 3).reshape(1024, 1024)
        Y[b, half * TC:(half + 1) * TC, :] = yc.T
    return Y


# revision 33
# speedup vs baseline: 1.0967x; 1.0967x over previous
"""Trainium2 Bass kernel for nn_LDS_LR: low-rank LDS + AR low-rank correction.

Math (per batch b):
    Bu   = X @ B1 @ B2                      # [T, N] rank-64 input projection
    h_t  = A * h_{t-1} + Bu_t               # diagonal recurrence, h_{-1} = h0
    lds  = H @ C1 @ C2                      # [T, O] rank-64 output projection
    proj = einsum('ti,rik->trk', X, M1)     # [T, R, KX]
    ar_t = sum_k M2[:,:,k] @ proj[t-k,:,k]  # AR with KX=5 taps
    Y    = lds + ar

Sharding: 8 cores = 4 batches x 2 sequence halves (1024 steps each).

v4 design notes (on top of v3's sorted-A windowed carries):
  * States host-permuted by |A| asc; windowed decay matmuls for the carry V
    and the CH1 corrections (~2k cols each instead of 8k).
  * Scan-then-correct at BOTH levels: block-1 scans start from zero, the
    missed A^(t'+1)*h_511 term is folded into CH1(1) as one more windowed
    matmul reusing the same apc slices.  All 16 scans are then independent:
    no serial 20us DVE chain; a few scans can offload to the Pool engine.
  * The carry-only inputs (xp, B1 copy, apv) travel as fp8e4m3 — the carry
    is a small additive term so 6% quantization there is ~0.3% on Y.  B1 is
    pre-scaled x1024 into fp8 range; the ones-reduction column carries the
    1/1024 compensation.
  * G matmul paired into PE column groups (rows 0:64 = tb0, 64:128 = tb1):
    half the column count and one eviction instead of two.
  * PE warm-up matmuls on a memset tile from t~0 keep the HAM clock-gate at
    2.4 GHz; input DMAs spread over the 3 hardware queues, k-loops consume
    in arrival order; xp scheduled mid-stream (carry chain needs it late).
"""

import contextlib
import ctypes
import os
import sys
import types

import numpy as np
from contextlib import ExitStack

import concourse.bass as bass
import concourse.tile as tile
from concourse import bacc, mybir
from concourse.bass_utils import run_bass_kernel_spmd


def _install_ntff_hook():
    """Provide antenv.axon_hooks.get_axon_ntff_profile_hook if the image
    lacks it, driving NTFF capture via the libaxon_pjrt C ABI directly."""
    try:
        from antenv.axon_hooks import get_axon_ntff_profile_hook  # noqa: F401
        return
    except ImportError:
        pass
    so_path = "/opt/axon/libaxon_pjrt.so"
    hook = None
    if os.path.exists(so_path):
        lib = ctypes.CDLL(so_path)
        if hasattr(lib, "axon_start_nrt_profile"):
            lib.axon_start_nrt_profile.argtypes = [
                ctypes.POINTER(ctypes.c_int64), ctypes.c_size_t]
            lib.axon_start_nrt_profile.restype = ctypes.c_int64
            lib.axon_stop_nrt_profile.argtypes = [ctypes.c_char_p]
            lib.axon_stop_nrt_profile.restype = ctypes.c_int64

            @contextlib.contextmanager
            def _hook(output_dir, device_ids):
                import jax
                jax.devices()
                if device_ids:
                    ids = (ctypes.c_int64 * len(device_ids))(*device_ids)
                    rc = lib.axon_start_nrt_profile(ids, len(device_ids))
                else:
                    rc = lib.axon_start_nrt_profile(None, 0)
                if rc != 0:
                    raise RuntimeError(f"axon_start_nrt_profile rc={rc}")
                try:
                    yield
                finally:
                    n = lib.axon_stop_nrt_profile(str(output_dir).encode())
                    print(f"ntff profile: {n} file(s) -> {output_dir}",
                          file=sys.stderr)

            hook = _hook
    mod = types.ModuleType("antenv.axon_hooks")
    mod.get_axon_ntff_profile_hook = lambda: hook
    mod.set_axon_ntff_profile_hook = lambda h: None
    sys.modules["antenv.axon_hooks"] = mod


_install_ntff_hook()

DT = mybir.dt.float32
MDT = mybir.dt.bfloat16
F8 = mybir.dt.float8e4
MNP = mybir.dt.np(MDT)
F8NP = mybir.dt.np(F8)
F32 = np.float32
ODT = MDT
ONP = mybir.dt.np(ODT)

B, T, D = 4, 2048, 1024
NST, R, KX, OUT = 1024, 64, 5, 1024
TC = 1024          # per-core chunk length
TBL = 512          # time block (one PSUM bank at fp32)

# decay windows per sorted n-tile (compile-time; states sorted by A asc)
APW = 2048         # apv/apc width: 1024 + 256 + 6*128
B2W = 1152         # b2x width: 1024 B2 + 64 ident + ones + pad

WARM_MM = int(os.environ.get("KERNEL_WARM_MM", "10"))
# scans whose Bu is staged PSUM->SBUF (ACT copy) so the DVE scan runs in the
# all-SBUF 2x mode; remaining scans read PSUM directly at 1x
STAGE_SCAN = int(os.environ.get("KERNEL_STAGE_SCAN", "0"))
USE_F8 = bool(int(os.environ.get("KERNEL_F8", "1")))      # fp8 carry path
# col-group-paired G matmul: faults trn2 hardware when the operands are fp8
# (bf16 pairing and unpaired fp8 both pass) — keep off
PAIR_G = bool(int(os.environ.get("KERNEL_PAIR_G", "0")))
F8D = F8 if USE_F8 else MDT   # dtype of the carry-path tensors
F8DNP = mybir.dt.np(F8D)

_CACHED_NC = None
LAST_RESULT = None  # BassKernelResults of the most recent run (for test.py)

MULT = mybir.AluOpType.mult
ADD = mybir.AluOpType.add

# xo k-tile consumption order ~ DMA arrival (q0,q2 sync / q1 gpsimd /
# q3 scalar-after-w1g)
KORD = [0, 4, 1, 5, 2, 6, 3, 7]


def _emit(ctx, tc, io):
    nc = tc.nc
    xo, xp, xtl, w1, w1g8, b2x, c1, w2, apv, apc, avio, yt = io

    wp = ctx.enter_context(tc.tile_pool(name="wp", bufs=1))
    xpool = ctx.enter_context(tc.tile_pool(name="xpool", bufs=1))
    hp = ctx.enter_context(tc.tile_pool(name="hp", bufs=1))
    pp = ctx.enter_context(tc.tile_pool(name="pp", bufs=1))
    yp = ctx.enter_context(tc.tile_pool(name="yp", bufs=1))
    pA = ctx.enter_context(tc.tile_pool(name="pA", bufs=3, space="PSUM"))
    pB = ctx.enter_context(tc.tile_pool(name="pB", bufs=3, space="PSUM"))
    pC = ctx.enter_context(tc.tile_pool(name="pC", bufs=2, space="PSUM"))

    # ---------------- warm-up: memset tile + dummy accumulating MMs ---------
    wtile = wp.tile([128, 640], MDT, tag="wtile", name="wtile")
    nc.gpsimd.memset(wtile[:], 0.0)
    wps = pC.tile([128, TBL], DT, tag="pc", name="wps")
    for i in range(WARM_MM):
        nc.tensor.matmul(wps[:], wtile[:, 512:640], wtile[:, 0:512],
                         start=(i == 0), stop=(i == WARM_MM - 1))

    # ---------------- input DMAs, 3 queues, deadline-ordered ----------------
    aviosb = wp.tile([128, 16], DT, tag="avio", name="aviosb")
    xots = [xpool.tile([128, 2048], MDT, tag=f"xoq{i}", name=f"xoq{i}")
            for i in range(4)]
    xpts = [xpool.tile([128, 2048], F8D, tag=f"xpq{i}", name=f"xpq{i}")
            for i in range(4)]
    c1sb = wp.tile([128, 512], MDT, tag="c1", name="c1sb")
    w1gsb = wp.tile([128, 1024], MDT, tag="w1g", name="w1gsb")
    w1rsb = wp.tile([128, 2048], MDT, tag="w1r", name="w1rsb")
    w1g8sb = wp.tile([128, 512], F8D, tag="w1g8", name="w1g8sb")
    xtlsb = wp.tile([128, 32], MDT, tag="xtl", name="xtlsb")
    b2xsb = wp.tile([128, B2W], MDT, tag="b2x", name="b2xsb")
    w2sb = wp.tile([128, 3072], MDT, tag="w2", name="w2sb")
    apvsb = wp.tile([128, APW], F8D, tag="apv", name="apvsb")
    apcsb = wp.tile([128, APW], MDT, tag="apc", name="apcsb")

    # sync queue: xo first, then xp quarter + corr weights
    nc.sync.dma_start(aviosb[:], avio[:])
    nc.sync.dma_start(xots[0][:], xo[:, 0:2048])
    nc.sync.dma_start(xots[1][:], xo[:, 2048:4096])
    nc.sync.dma_start(xpts[0][:], xp[:, 0:2048])
    nc.sync.dma_start(c1sb[:], c1[:])
    nc.sync.dma_start(apcsb[:], apc[:])
    # gpsimd queue: xo second half, then xp
    nc.gpsimd.dma_start(xots[2][:], xo[:, 4096:6144])
    nc.gpsimd.dma_start(xots[3][:], xo[:, 6144:8192])
    nc.gpsimd.dma_start(xpts[1][:], xp[:, 2048:4096])
    nc.gpsimd.dma_start(xpts[2][:], xp[:, 4096:6144])
    nc.gpsimd.dma_start(xpts[3][:], xp[:, 6144:8192])
    # scalar queue: all weights
    nc.scalar.dma_start(w1gsb[:], w1[:, 0:1024])
    nc.scalar.dma_start(w1g8sb[:], w1g8[:])
    nc.scalar.dma_start(b2xsb[:], b2x[:])
    nc.scalar.dma_start(w1rsb[:], w1[:, 1024:3072])
    nc.scalar.dma_start(xtlsb[:], xtl[:])
    nc.scalar.dma_start(apvsb[:], apv[:])
    nc.scalar.dma_start(w2sb[:], w2[:])

    def xot(k):
        return xots[k // 2][:, (k % 2) * 1024:(k % 2 + 1) * 1024]

    def xpt(k):
        return xpts[k // 2][:, (k % 2) * 1024:(k % 2 + 1) * 1024]

    def w1t(k, lo, hi):  # W1 k-tile column slice (w1g: cols 0:128, w1r: rest)
        if hi <= 128:
            return w1gsb[:, k * 128 + lo:k * 128 + hi]
        assert lo >= 128
        return w1rsb[:, k * 256 + lo - 128:k * 256 + hi - 128]

    def w2t(m, o):  # W2 stationary for (m-tile, o-tile) [128, 128]
        return w2sb[:, m * 1024 + o * 128:m * 1024 + (o + 1) * 128]

    def abv(n):  # A broadcast for scans, stride-0 partition view
        return aviosb[:, n:n + 1].broadcast_to((128, TBL))

    # ---------------- j0 = [B1|tap0]^T Xo -----------------------------------
    PW = 4 + TC + 4
    pext = [pp.tile([128, PW], MDT, tag=f"pext{j}", name=f"pext{j}")
            for j in range(3)]
    j0ps = [pA.tile([128, TBL], DT, tag="pa", name=f"j0_ps{t}")
            for t in range(2)]
    for i, k in enumerate(KORD):
        for t in range(2):
            nc.tensor.matmul(j0ps[t][:], w1t(k, 0, 128),
                             xot(k)[:, t * TBL:(t + 1) * TBL],
                             start=(i == 0), stop=(i == 7))
    for t in range(2):
        nc.scalar.copy(pext[0][:, 4 + t * TBL:4 + (t + 1) * TBL], j0ps[t][:])

    # ---------------- Bu + 16 independent scans -----------------------------
    # b0 scans start from the host-folded h0 offset; b1 scans start from 0 and
    # the missed A^(t'+1)*h_511 term lands in CH1(1) via the sc2 correction.
    hsb = [hp.tile([128, TC], MDT, tag=f"h{n}", name=f"h{n}") for n in range(8)]

    def emit_buo(n, tb):
        bu = pB.tile([128, TBL], DT, tag="pb", name=f"buo{n}_{tb}")
        nc.tensor.matmul(bu[:], b2xsb[0:64, n * 128:(n + 1) * 128],
                         pext[0][0:64, 4 + tb * TBL:4 + (tb + 1) * TBL],
                         start=True, stop=True)
        init = aviosb[:, 8 + n:9 + n] if tb == 0 else 0.0
        dst = hsb[n][:, tb * TBL:(tb + 1) * TBL]
        nc.vector.tensor_tensor_scan(dst, abv(n), bu[:], init, MULT, ADD)

    emit_buo(0, 0)
    emit_buo(1, 0)

    # ---------------- carry chain: G = B1^T Xp, G^T, V, E, D ----------------
    gsb = wp.tile([64, 1024], MDT, tag="gprev", name="gsb")
    g_ps = [pC.tile([64, TBL], DT, tag="pc", name=f"g_ps{t}")
            for t in range(2)]
    for i, k in enumerate(range(8)):
        st = w1g8sb[:, k * 64:(k + 1) * 64]
        for t in range(2):
            nc.tensor.matmul(g_ps[t][:], st,
                             xpt(k)[:, t * TBL:(t + 1) * TBL],
                             start=(i == 0), stop=(i == 7))
    for t in range(2):
        nc.scalar.copy(gsb[:, t * TBL:(t + 1) * TBL], g_ps[t][:])

    emit_buo(2, 0)

    # gt[:, st*64:(st+1)*64] = (G[:, st*128:(st+1)*128])^T via identity MMs
    gtsb = wp.tile([128, 512], F8D, tag="gt", name="gtsb")
    gt_ps = pC.tile([128, 512], DT, tag="pc", name="gt_ps")
    for st in range(8):
        nc.tensor.matmul(gt_ps[:, st * 64:(st + 1) * 64],
                         gsb[:, st * 128:(st + 1) * 128],
                         b2xsb[0:64, 1024:1088], start=True, stop=True)
    nc.scalar.copy(gtsb[:], gt_ps[:])

    emit_buo(3, 0)
    emit_buo(4, 0)

    # ---------------- j1 / j2 (with xtl-fed boundary tails) -----------------
    def emit_j(j, klo, khi, jps, tl):
        for i in range(klo, khi):
            k = KORD[i]
            st = w1t(k, j * 128, (j + 1) * 128)
            for t in range(2):
                nc.tensor.matmul(jps[t][:], st,
                                 xot(k)[:, t * TBL:(t + 1) * TBL],
                                 start=(i == 0), stop=(i == 7))
            nc.tensor.matmul(tl[:], st, xtlsb[:, k * 4:(k + 1) * 4],
                             start=(i == 0), stop=(i == 7))

    def evict_j(j, jps, tl):
        ka, kb = 2 * j - 1, 2 * j
        for t in range(2):
            nc.scalar.copy(
                pext[j][0:64, 4 + ka + t * TBL:4 + ka + (t + 1) * TBL],
                jps[t][0:64, :])
            nc.scalar.copy(
                pext[j][64:128, 4 + kb + t * TBL:4 + kb + (t + 1) * TBL],
                jps[t][64:128, :])
        nc.scalar.copy(pext[j][0:64, 4:4 + ka], tl[0:64, 4 - ka:4])
        nc.scalar.copy(pext[j][64:128, 4:4 + kb], tl[64:128, 4 - kb:4])

    j1ps = [pC.tile([128, TBL], DT, tag="pc", name=f"j1_ps{t}")
            for t in range(2)]
    tl1 = pA.tile([128, 4], DT, tag="pa", name="tl1")
    emit_j(1, 0, 4, j1ps, tl1)
    emit_buo(5, 0)

    # V[r, n] = sum_s G[r, s] A[n]^(1023-s), windowed by sorted-A tiles.
    # apv segments: [0:1024]=st7 all n; [1024:1280]=st6 n 768:1024;
    # [1280+st*128 ...]=st 0..5, n 896:1024.
    v_ps = [pC.tile([64, TBL], DT, tag="pc", name=f"v_ps{nh}")
            for nh in range(2)]
    nc.tensor.matmul(v_ps[0][:], gtsb[:, 448:512], apvsb[:, 0:512],
                     start=True, stop=True)
    nc.tensor.matmul(v_ps[1][:], gtsb[:, 448:512], apvsb[:, 512:1024],
                     start=True, stop=False)
    nc.tensor.matmul(v_ps[1][:, 256:512], gtsb[:, 384:448],
                     apvsb[:, 1024:1280], start=False, stop=False)
    for st in range(6):
        nc.tensor.matmul(v_ps[1][:, 384:512], gtsb[:, st * 64:(st + 1) * 64],
                         apvsb[:, 1280 + st * 128:1280 + (st + 1) * 128],
                         start=False, stop=(st == 5))

    # E = V * B2 elementwise (DVE — slots between scans 5 and 6);
    # D[n] = sum_r E[r, n] * (1/1024 ones-matmul)
    esb = wp.tile([64, 1024], MDT, tag="esb", name="esb")
    for nh in range(2):
        nc.vector.scalar_tensor_tensor(
            esb[:, nh * TBL:(nh + 1) * TBL], v_ps[nh][:], 1.0,
            b2xsb[0:64, nh * TBL:(nh + 1) * TBL], MULT, MULT)
    d_ps = pA.tile([128, 8], DT, tag="pa", name="d_ps")
    for n in range(8):
        nc.tensor.matmul(d_ps[:, n:n + 1], esb[:, n * 128:(n + 1) * 128],
                         b2xsb[0:64, 1088:1089], start=True, stop=True)

    emit_buo(6, 0)
    emit_j(1, 4, 8, j1ps, tl1)
    evict_j(1, j1ps, tl1)
    emit_buo(7, 0)

    # correction stationaries on the ACT engine: scorr = C1*D (chunk carry),
    # sc2 = C1*h_511 (block-1 zero-init carry).  Emitted after ALL b0 scans
    # so the hsb reads depend on the scan writes.
    scorr = wp.tile([128, 512], MDT, tag="scorr", name="scorr")
    sc2 = wp.tile([128, 512], MDT, tag="sc2", name="sc2")
    h511f = wp.tile([128, 8], DT, tag="h511f", name="h511f")
    dsb = wp.tile([128, 8], DT, tag="dsb", name="dsb")
    nc.scalar.copy(dsb[:], d_ps[:])
    for nt in range(8):
        nc.scalar.mul(scorr[:, nt * 64:(nt + 1) * 64],
                      c1sb[:, nt * 64:(nt + 1) * 64], dsb[:, nt:nt + 1])
    for nt in range(8):
        nc.scalar.copy(h511f[:, nt:nt + 1], hsb[nt][:, TBL - 1:TBL])
        nc.scalar.mul(sc2[:, nt * 64:(nt + 1) * 64],
                      c1sb[:, nt * 64:(nt + 1) * 64], h511f[:, nt:nt + 1])

    j2ps = [pC.tile([128, TBL], DT, tag="pc", name=f"j2_ps{t}")
            for t in range(2)]
    tl2 = pA.tile([128, 4], DT, tag="pa", name="tl2")
    emit_j(2, 0, 4, j2ps, tl2)
    emit_buo(0, 1)
    emit_j(2, 4, 8, j2ps, tl2)
    evict_j(2, j2ps, tl2)
    emit_buo(1, 1)

    # ---------------- CH1(tb) + windowed corrections, Y(tb) -----------------
    ysb = [[yp.tile([128, 4 * TBL], ODT, tag=f"y{tb}{g}", name=f"y{tb}{g}")
            for g in range(2)] for tb in range(2)]

    def corr_mms(cps, stat, last_stop):
        # windowed A^(t+1) correction: tile7 full 512, tile6 256, rest 128
        nc.tensor.matmul(cps[:], stat[:, 448:512], apcsb[:, 0:512],
                         start=False, stop=False)
        nc.tensor.matmul(cps[:, 0:256], stat[:, 384:448],
                         apcsb[:, 1024:1280], start=False, stop=False)
        for nt in range(6):
            nc.tensor.matmul(
                cps[:, 0:128], stat[:, nt * 64:(nt + 1) * 64],
                apcsb[:, 1280 + nt * 128:1280 + (nt + 1) * 128],
                start=False, stop=(last_stop and nt == 5))

    def emit_ch1(tb):
        cps = pC.tile([64, TBL], DT, tag="pc", name=f"c_ps{tb}")
        for n in range(8):
            nc.tensor.matmul(cps[:], c1sb[:, n * 64:(n + 1) * 64],
                             hsb[n][:, tb * TBL:(tb + 1) * TBL],
                             start=(n == 0), stop=False)
        if tb == 0:
            corr_mms(cps, scorr, True)
        else:
            # chunk carry at t 512:1024 decays below cutoff except tile 7
            nc.tensor.matmul(cps[:], scorr[:, 448:512], apcsb[:, 512:1024],
                             start=False, stop=False)
            corr_mms(cps, sc2, True)
        nc.scalar.copy(pext[0][0:64, 4 + tb * TBL:4 + (tb + 1) * TBL], cps[:])

    # Y: per o-tile accumulate m=1, m=2 early; the CH1-dependent m=0 last.
    yq = {}

    def y_mm12(tb, o):
        yps = pA.tile([128, TBL], DT, tag="pa", name=f"y_ps{o}_{tb}")
        yq[(tb, o)] = yps
        nc.tensor.matmul(yps[:], w2t(1, o),
                         pext[1][:, 4 + tb * TBL:4 + (tb + 1) * TBL],
                         start=True, stop=False)
        nc.tensor.matmul(yps[:], w2t(2, o),
                         pext[2][:, 4 + tb * TBL:4 + (tb + 1) * TBL],
                         start=False, stop=False)

    def y_mm0(tb, o):
        yps = yq.pop((tb, o))
        nc.tensor.matmul(yps[:], w2t(0, o),
                         pext[0][:, 4 + tb * TBL:4 + (tb + 1) * TBL],
                         start=False, stop=True)
        g, oo = divmod(o, 4)
        dst = ysb[tb][g][:, oo * TBL:(oo + 1) * TBL]
        if (tb == 0 and o < 4) or (tb == 1 and o % 2 == 1):
            nc.scalar.copy(dst, yps[:])
        else:
            nc.vector.tensor_copy(dst, yps[:])
        if oo == 3:
            eng = [nc.sync, nc.gpsimd, nc.gpsimd, nc.sync][tb * 2 + g]
            eng.dma_start(
                yt[:, tb * 4096 + g * 2048:tb * 4096 + (g + 1) * 2048],
                ysb[tb][g][:])

    emit_ch1(0)
    emit_buo(2, 1)

    # CH1(1): n-matmuls interleaved into the Y(0) pipeline (each waits only
    # its own b1 scan); the s(7,1)-dependent pieces + corrections come last.
    cps1 = pC.tile([64, TBL], DT, tag="pc", name="c_ps1")

    def ch1_1_n(n, start=False):
        nc.tensor.matmul(cps1[:], c1sb[:, n * 64:(n + 1) * 64],
                         hsb[n][:, TBL:TC], start=start, stop=False)

    ch1_1_n(0, start=True)
    y_mm12(0, 0)
    y_mm12(0, 1)
    y_mm12(0, 2)
    y_mm0(0, 0)
    emit_buo(3, 1)
    ch1_1_n(1)
    y_mm12(0, 3)
    y_mm0(0, 1)
    y_mm12(0, 4)
    y_mm0(0, 2)
    emit_buo(4, 1)
    ch1_1_n(2)
    y_mm12(0, 5)
    y_mm0(0, 3)
    y_mm12(0, 6)
    y_mm0(0, 4)
    emit_buo(5, 1)
    ch1_1_n(3)
    y_mm12(0, 7)
    y_mm0(0, 5)
    y_mm0(0, 6)
    emit_buo(6, 1)
    ch1_1_n(4)
    y_mm0(0, 7)
    y_mm12(1, 0)
    emit_buo(7, 1)
    ch1_1_n(5)
    y_mm12(1, 1)
    y_mm12(1, 2)
    ch1_1_n(6)
    ch1_1_n(7)
    nc.tensor.matmul(cps1[:], scorr[:, 448:512], apcsb[:, 512:1024],
                     start=False, stop=False)
    corr_mms(cps1, sc2, True)
    nc.vector.tensor_copy(pext[0][0:64, 4 + TBL:4 + TC], cps1[:])
    y_mm0(1, 0)
    y_mm12(1, 3)
    y_mm0(1, 1)
    y_mm12(1, 4)
    y_mm0(1, 2)
    y_mm12(1, 5)
    y_mm0(1, 3)
    y_mm12(1, 6)
    y_mm0(1, 4)
    y_mm12(1, 7)
    y_mm0(1, 5)
    y_mm0(1, 6)
    y_mm0(1, 7)


def _build():
    nc = bacc.Bacc("TRN2", target_bir_lowering=False, debug=False,
                   num_devices=8)
    xo = nc.dram_tensor("xo", [128, 8192], MDT, kind="ExternalInput").ap()
    xp = nc.dram_tensor("xp", [128, 8192], F8D, kind="ExternalInput").ap()
    xtl = nc.dram_tensor("xtl", [128, 32], MDT, kind="ExternalInput").ap()
    w1 = nc.dram_tensor("w1", [128, 3072], MDT, kind="ExternalInput").ap()
    w1g8 = nc.dram_tensor("w1g8", [128, 512], F8D, kind="ExternalInput").ap()
    b2x = nc.dram_tensor("b2x", [128, B2W], MDT, kind="ExternalInput").ap()
    c1 = nc.dram_tensor("c1", [128, 512], MDT, kind="ExternalInput").ap()
    w2 = nc.dram_tensor("w2", [128, 3072], MDT, kind="ExternalInput").ap()
    apv = nc.dram_tensor("apv", [128, APW], F8D, kind="ExternalInput").ap()
    apc = nc.dram_tensor("apc", [128, APW], MDT, kind="ExternalInput").ap()
    avio = nc.dram_tensor("avio", [128, 16], DT, kind="ExternalInput").ap()
    yt = nc.dram_tensor("yt", [128, 8192], ODT, kind="ExternalOutput").ap()

    with tile.TileContext(nc) as tc, ExitStack() as ctx:
        _emit(ctx, tc, (xo, xp, xtl, w1, w1g8, b2x, c1, w2, apv, apc,
                        avio, yt))
    nc.compile()
    return nc


def _get_nc():
    global _CACHED_NC
    if _CACHED_NC is None:
        _CACHED_NC = _build()
    return _CACHED_NC


def _pack_kt(arr):
    """[1024, C] -> [128, 8*C] with blocks of 128 rows side by side."""
    C = arr.shape[1]
    return np.ascontiguousarray(
        arr.reshape(8, 128, C).transpose(1, 0, 2).reshape(128, 8 * C))


def kernel(inputs, h0, A, B1, B2, C1, C2, M1, M2):
    global LAST_RESULT
    X = np.asarray(inputs, dtype=F32)
    h0 = np.asarray(h0, dtype=F32)
    A = np.asarray(A, dtype=F32)

    # sort states by A ascending (weights-only permutation)
    perm = np.argsort(np.asarray(A, dtype=np.float64))
    As64 = np.asarray(A, dtype=np.float64)[perm]
    h0s = h0[perm]
    B2s = np.asarray(B2, dtype=F32)[:, perm]
    C1s = np.asarray(C1, dtype=F32)[perm, :]

    # sanity: windows hold for this A draw (program structure is fixed)
    assert As64[767] ** 128 < 1e-4, As64[767]
    assert As64[895] ** 256 < 1e-4, As64[895]

    W1 = np.concatenate(
        [np.asarray(B1, dtype=F32)]
        + [np.ascontiguousarray(np.asarray(M1, dtype=F32)[:, :, k].T)
           for k in range(KX)], axis=1)
    W2 = np.concatenate(
        [np.asarray(C2, dtype=F32)]
        + [np.ascontiguousarray(np.asarray(M2, dtype=F32)[:, :, k].T)
           for k in range(KX)], axis=0)
    w1kt = W1.astype(MNP).reshape(8, 128, 384)           # [k, p, c]
    w1p = np.concatenate([                               # [128, 1024 | 2048]
        np.ascontiguousarray(w1kt[:, :, 0:128].transpose(1, 0, 2)
                             .reshape(128, 1024)),
        np.ascontiguousarray(w1kt[:, :, 128:384].transpose(1, 0, 2)
                             .reshape(128, 2048))], axis=1)
    # fp8 B1 copy, pre-scaled x1024 into e4m3 range (G path only)
    b1s = (np.asarray(B1, dtype=F32) * 1024.0).astype(F8DNP)  # [1024, 64]
    w1g8p = np.ascontiguousarray(
        b1s.reshape(8, 128, 64).transpose(1, 0, 2).reshape(128, 512))
    w2p = np.ascontiguousarray(
        W2.reshape(3, 128, 1024).transpose(1, 0, 2)
        .reshape(128, 3072).astype(MNP))                 # [128, 3072]
    b2xm = np.zeros((128, B2W), F32)
    b2xm[0:64, 0:1024] = B2s
    b2xm[0:64, 1024:1088] = np.eye(64, dtype=F32)
    b2xm[64:128, 1024:1088] = np.eye(64, dtype=F32)
    b2xm[0:64, 1088] = 1.0 / 1024.0
    b2xp = np.ascontiguousarray(b2xm.astype(MNP))
    c1p = _pack_kt(C1s.astype(MNP))                      # [128, 512]

    # apv: V decay powers A^(1023-s), windowed; s = st*128 + p.
    lnAs = np.log(As64)
    p = np.arange(128, dtype=np.float64)
    apvm = np.zeros((128, APW), np.float64)
    apvm[:, 0:1024] = np.exp(np.outer(127.0 - p, lnAs))              # st7
    apvm[:, 1024:1280] = np.exp(np.outer(255.0 - p, lnAs[768:1024]))  # st6
    for st in range(6):
        apvm[:, 1280 + st * 128:1280 + (st + 1) * 128] = np.exp(
            np.outer(1023.0 - (st * 128 + p), lnAs[896:1024]))
    apvp = np.ascontiguousarray(apvm.astype(F32).astype(F8DNP))

    # apc: correction decay powers A^(t+1), windowed per sorted n-tile.
    t1 = np.arange(1, 1025, dtype=np.float64)
    apcm = np.zeros((128, APW), np.float64)
    apcm[:, 0:1024] = np.exp(np.outer(lnAs[896:1024], t1))           # tile7
    apcm[:, 1024:1280] = np.exp(np.outer(lnAs[768:896], t1[0:256]))  # tile6
    for nt in range(6):
        apcm[:, 1280 + nt * 128:1280 + (nt + 1) * 128] = np.exp(
            np.outer(lnAs[nt * 128:(nt + 1) * 128], t1[0:128]))
    apcp = np.ascontiguousarray(apcm.astype(F32).astype(MNP))

    ioff_h0 = h0s.astype(F32)                              # half 0: plain h0
    ioff_h1 = (As64 ** TC * h0s.astype(np.float64)).astype(F32)  # A^1024 h0

    Xbf = X.astype(MNP)
    X8 = X.astype(F8DNP)
    zeros_xp = np.zeros((128, 8192), F8DNP)
    zeros_xtl = np.zeros((128, 32), MNP)

    def pack_x(xarr, b, sl):
        return _pack_kt(np.ascontiguousarray(xarr[b, sl, :].T))

    in_maps = []
    for c in range(8):
        b, half = divmod(c, 2)
        xoc = pack_x(Xbf, b, slice(half * TC, (half + 1) * TC))
        if half == 0:
            xpc, xtlc, ioff = zeros_xp, zeros_xtl, ioff_h0
        else:
            xpc = pack_x(X8, b, slice(0, TC))
            xpbf = pack_x(Xbf, b, slice(0, TC))
            # xtl: last 4 time-cols of each k-tile of xp, [128, 8*4]
            xtlc = np.ascontiguousarray(
                xpbf.reshape(128, 8, 1024)[:, :, 1020:1024].reshape(128, 32))
            ioff = ioff_h1
        aviom = np.zeros((128, 16), F32)
        aviom[:, 0:8] = As64.astype(F32).reshape(8, 128).T
        aviom[:, 8:16] = ioff.reshape(8, 128).T
        in_maps.append({"xo": xoc, "xp": xpc, "xtl": xtlc, "w1": w1p,
                        "w1g8": w1g8p, "b2x": b2xp, "c1": c1p, "w2": w2p,
                        "apv": apvp, "apc": apcp, "avio": aviom})

    nc = _get_nc()
    trace = bool(int(os.environ.get("KERNEL_TRACE", "0")))
    LAST_RESULT = run_bass_kernel_spmd(nc, in_maps, core_ids=list(range(8)),
                                       trace=trace)
    Y = np.empty((B, T, OUT), F32)
    for c in range(8):
        b, half = divmod(c, 2)
        ytc = np.asarray(LAST_RESULT.results[c]["yt"], dtype=F32)
        # yt[p, tb*4096 + o*512 + t] -> Y_core[o*128+p, tb*512+t]
        yc = ytc.reshape(128, 2, 8, 512).transpose(2, 0, 1, 3).reshape(1024, 1024)
        Y[b, half * TC:(half + 1) * TC, :] = yc.T
    return Y


# revision 34
# speedup vs baseline: 1.1054x; 1.0079x over previous
"""Trainium2 Bass kernel for nn_LDS_LR: low-rank LDS + AR low-rank correction.

Math (per batch b):
    Bu   = X @ B1 @ B2                      # [T, N] rank-64 input projection
    h_t  = A * h_{t-1} + Bu_t               # diagonal recurrence, h_{-1} = h0
    lds  = H @ C1 @ C2                      # [T, O] rank-64 output projection
    proj = einsum('ti,rik->trk', X, M1)     # [T, R, KX]
    ar_t = sum_k M2[:,:,k] @ proj[t-k,:,k]  # AR with KX=5 taps
    Y    = lds + ar

Sharding: 8 cores = 4 batches x 2 sequence halves (1024 steps each).

v4 design notes (on top of v3's sorted-A windowed carries):
  * States host-permuted by |A| asc; windowed decay matmuls for the carry V
    and the CH1 corrections (~2k cols each instead of 8k).
  * Scan-then-correct at BOTH levels: block-1 scans start from zero, the
    missed A^(t'+1)*h_511 term is folded into CH1(1) as one more windowed
    matmul reusing the same apc slices.  All 16 scans are then independent:
    no serial 20us DVE chain; a few scans can offload to the Pool engine.
  * The carry-only inputs (xp, B1 copy, apv) travel as fp8e4m3 — the carry
    is a small additive term so 6% quantization there is ~0.3% on Y.  B1 is
    pre-scaled x1024 into fp8 range; the ones-reduction column carries the
    1/1024 compensation.
  * G matmul paired into PE column groups (rows 0:64 = tb0, 64:128 = tb1):
    half the column count and one eviction instead of two.
  * PE warm-up matmuls on a memset tile from t~0 keep the HAM clock-gate at
    2.4 GHz; input DMAs spread over the 3 hardware queues, k-loops consume
    in arrival order; xp scheduled mid-stream (carry chain needs it late).
"""

import contextlib
import ctypes
import os
import sys
import types

import numpy as np
from contextlib import ExitStack

import concourse.bass as bass
import concourse.tile as tile
from concourse import bacc, mybir
from concourse.bass_utils import run_bass_kernel_spmd


def _install_ntff_hook():
    """Provide antenv.axon_hooks.get_axon_ntff_profile_hook if the image
    lacks it, driving NTFF capture via the libaxon_pjrt C ABI directly."""
    try:
        from antenv.axon_hooks import get_axon_ntff_profile_hook  # noqa: F401
        return
    except ImportError:
        pass
    so_path = "/opt/axon/libaxon_pjrt.so"
    hook = None
    if os.path.exists(so_path):
        lib = ctypes.CDLL(so_path)
        if hasattr(lib, "axon_start_nrt_profile"):
            lib.axon_start_nrt_profile.argtypes = [
                ctypes.POINTER(ctypes.c_int64), ctypes.c_size_t]
            lib.axon_start_nrt_profile.restype = ctypes.c_int64
            lib.axon_stop_nrt_profile.argtypes = [ctypes.c_char_p]
            lib.axon_stop_nrt_profile.restype = ctypes.c_int64

            @contextlib.contextmanager
            def _hook(output_dir, device_ids):
                import jax
                jax.devices()
                if device_ids:
                    ids = (ctypes.c_int64 * len(device_ids))(*device_ids)
                    rc = lib.axon_start_nrt_profile(ids, len(device_ids))
                else:
                    rc = lib.axon_start_nrt_profile(None, 0)
                if rc != 0:
                    raise RuntimeError(f"axon_start_nrt_profile rc={rc}")
                try:
                    yield
                finally:
                    n = lib.axon_stop_nrt_profile(str(output_dir).encode())
                    print(f"ntff profile: {n} file(s) -> {output_dir}",
                          file=sys.stderr)

            hook = _hook
    mod = types.ModuleType("antenv.axon_hooks")
    mod.get_axon_ntff_profile_hook = lambda: hook
    mod.set_axon_ntff_profile_hook = lambda h: None
    sys.modules["antenv.axon_hooks"] = mod


_install_ntff_hook()

DT = mybir.dt.float32
MDT = mybir.dt.bfloat16
F8 = mybir.dt.float8e4
MNP = mybir.dt.np(MDT)
F8NP = mybir.dt.np(F8)
F32 = np.float32
ODT = MDT
ONP = mybir.dt.np(ODT)

B, T, D = 4, 2048, 1024
NST, R, KX, OUT = 1024, 64, 5, 1024
TC = 1024          # per-core chunk length
TBL = 512          # time block (one PSUM bank at fp32)

# decay windows per sorted n-tile (compile-time; states sorted by A asc)
APW = 2048         # apv/apc width: 1024 + 256 + 6*128
B2W = 1152         # b2x width: 1024 B2 + 64 ident + ones + pad

WARM_MM = int(os.environ.get("KERNEL_WARM_MM", "10"))
# scans whose Bu is staged PSUM->SBUF (ACT copy) so the DVE scan runs in the
# all-SBUF 2x mode; remaining scans read PSUM directly at 1x
STAGE_SCAN = int(os.environ.get("KERNEL_STAGE_SCAN", "0"))
USE_F8 = bool(int(os.environ.get("KERNEL_F8", "1")))      # fp8 carry path
# col-group-paired G matmul: faults trn2 hardware when the operands are fp8
# (bf16 pairing and unpaired fp8 both pass) — keep off
PAIR_G = bool(int(os.environ.get("KERNEL_PAIR_G", "0")))
F8D = F8 if USE_F8 else MDT   # dtype of the carry-path tensors
F8DNP = mybir.dt.np(F8D)

_CACHED_NC = None
LAST_RESULT = None  # BassKernelResults of the most recent run (for test.py)

MULT = mybir.AluOpType.mult
ADD = mybir.AluOpType.add

# xo k-tile consumption order ~ DMA arrival (q0,q2 sync / q1 gpsimd /
# q3 scalar-after-w1g)
KORD = [0, 4, 1, 5, 2, 6, 3, 7]


def _emit(ctx, tc, io):
    nc = tc.nc
    xo, xp, xtl, w1, w1g8, b2x, c1, w2, apv, apc, avio, yt = io

    wp = ctx.enter_context(tc.tile_pool(name="wp", bufs=1))
    xpool = ctx.enter_context(tc.tile_pool(name="xpool", bufs=1))
    hp = ctx.enter_context(tc.tile_pool(name="hp", bufs=1))
    pp = ctx.enter_context(tc.tile_pool(name="pp", bufs=1))
    yp = ctx.enter_context(tc.tile_pool(name="yp", bufs=1))
    pA = ctx.enter_context(tc.tile_pool(name="pA", bufs=3, space="PSUM"))
    pB = ctx.enter_context(tc.tile_pool(name="pB", bufs=3, space="PSUM"))
    pC = ctx.enter_context(tc.tile_pool(name="pC", bufs=2, space="PSUM"))

    # ---------------- warm-up: memset tile + dummy accumulating MMs ---------
    wtile = wp.tile([128, 640], MDT, tag="wtile", name="wtile")
    nc.gpsimd.memset(wtile[:], 0.0)
    wps = pC.tile([128, TBL], DT, tag="pc", name="wps")
    for i in range(WARM_MM):
        nc.tensor.matmul(wps[:], wtile[:, 512:640], wtile[:, 0:512],
                         start=(i == 0), stop=(i == WARM_MM - 1))

    # ---------------- input DMAs, 3 queues, deadline-ordered ----------------
    aviosb = wp.tile([128, 16], DT, tag="avio", name="aviosb")
    xots = [xpool.tile([128, 2048], MDT, tag=f"xoq{i}", name=f"xoq{i}")
            for i in range(4)]
    xpts = [xpool.tile([128, 2048], F8D, tag=f"xpq{i}", name=f"xpq{i}")
            for i in range(4)]
    c1sb = wp.tile([128, 512], MDT, tag="c1", name="c1sb")
    w1gsb = wp.tile([128, 1024], MDT, tag="w1g", name="w1gsb")
    w1rsb = wp.tile([128, 2048], MDT, tag="w1r", name="w1rsb")
    w1g8sb = wp.tile([128, 512], F8D, tag="w1g8", name="w1g8sb")
    xtlsb = wp.tile([128, 32], MDT, tag="xtl", name="xtlsb")
    b2xsb = wp.tile([128, B2W], MDT, tag="b2x", name="b2xsb")
    w2sb = wp.tile([128, 3072], MDT, tag="w2", name="w2sb")
    apvsb = wp.tile([128, APW], F8D, tag="apv", name="apvsb")
    apcsb = wp.tile([128, APW], MDT, tag="apc", name="apcsb")

    # sync queue: xo first, then xp quarter + corr weights
    nc.sync.dma_start(aviosb[:], avio[:])
    nc.sync.dma_start(xots[0][:], xo[:, 0:2048])
    nc.sync.dma_start(xots[1][:], xo[:, 2048:4096])
    nc.sync.dma_start(xpts[0][:], xp[:, 0:2048])
    nc.sync.dma_start(c1sb[:], c1[:])
    nc.sync.dma_start(apcsb[:], apc[:])
    # gpsimd queue: xo second half, then xp
    nc.gpsimd.dma_start(xots[2][:], xo[:, 4096:6144])
    nc.gpsimd.dma_start(xots[3][:], xo[:, 6144:8192])
    nc.gpsimd.dma_start(xpts[1][:], xp[:, 2048:4096])
    nc.gpsimd.dma_start(xpts[2][:], xp[:, 4096:6144])
    nc.gpsimd.dma_start(xpts[3][:], xp[:, 6144:8192])
    # scalar queue: all weights
    nc.scalar.dma_start(w1gsb[:], w1[:, 0:1024])
    nc.scalar.dma_start(w1g8sb[:], w1g8[:])
    nc.scalar.dma_start(b2xsb[:], b2x[:])
    nc.scalar.dma_start(w1rsb[:], w1[:, 1024:3072])
    nc.scalar.dma_start(xtlsb[:], xtl[:])
    nc.scalar.dma_start(apvsb[:], apv[:])
    nc.scalar.dma_start(w2sb[:], w2[:])

    def xot(k):
        return xots[k // 2][:, (k % 2) * 1024:(k % 2 + 1) * 1024]

    def xpt(k):
        return xpts[k // 2][:, (k % 2) * 1024:(k % 2 + 1) * 1024]

    def w1t(k, lo, hi):  # W1 k-tile column slice (w1g: cols 0:128, w1r: rest)
        if hi <= 128:
            return w1gsb[:, k * 128 + lo:k * 128 + hi]
        assert lo >= 128
        return w1rsb[:, k * 256 + lo - 128:k * 256 + hi - 128]

    def w2t(m, o):  # W2 stationary for (m-tile, o-tile) [128, 128]
        return w2sb[:, m * 1024 + o * 128:m * 1024 + (o + 1) * 128]

    def abv(n):  # A broadcast for scans, stride-0 partition view
        return aviosb[:, n:n + 1].broadcast_to((128, TBL))

    # ---------------- j0 = [B1|tap0]^T Xo -----------------------------------
    PW = 4 + TC + 4
    pext = [pp.tile([128, PW], MDT, tag=f"pext{j}", name=f"pext{j}")
            for j in range(3)]
    j0ps = [pA.tile([128, TBL], DT, tag="pa", name=f"j0_ps{t}")
            for t in range(2)]
    for i, k in enumerate(KORD):
        for t in range(2):
            nc.tensor.matmul(j0ps[t][:], w1t(k, 0, 128),
                             xot(k)[:, t * TBL:(t + 1) * TBL],
                             start=(i == 0), stop=(i == 7))
    for t in range(2):
        nc.scalar.copy(pext[0][:, 4 + t * TBL:4 + (t + 1) * TBL], j0ps[t][:])

    # ---------------- Bu + 16 independent scans -----------------------------
    # b0 scans start from the host-folded h0 offset; b1 scans start from 0 and
    # the missed A^(t'+1)*h_511 term lands in CH1(1) via the sc2 correction.
    hsb = [hp.tile([128, TC], MDT, tag=f"h{n}", name=f"h{n}") for n in range(8)]

    def emit_buo(n, tb):
        bu = pB.tile([128, TBL], DT, tag="pb", name=f"buo{n}_{tb}")
        nc.tensor.matmul(bu[:], b2xsb[0:64, n * 128:(n + 1) * 128],
                         pext[0][0:64, 4 + tb * TBL:4 + (tb + 1) * TBL],
                         start=True, stop=True)
        init = aviosb[:, 8 + n:9 + n] if tb == 0 else 0.0
        dst = hsb[n][:, tb * TBL:(tb + 1) * TBL]
        nc.vector.tensor_tensor_scan(dst, abv(n), bu[:], init, MULT, ADD)

    emit_buo(0, 0)
    emit_buo(1, 0)

    # ---------------- carry chain: G = B1^T Xp, G^T, V, E, D ----------------
    gsb = wp.tile([64, 1024], MDT, tag="gprev", name="gsb")
    g_ps = [pC.tile([64, TBL], DT, tag="pc", name=f"g_ps{t}")
            for t in range(2)]
    for i, k in enumerate(range(8)):
        st = w1g8sb[:, k * 64:(k + 1) * 64]
        for t in range(2):
            nc.tensor.matmul(g_ps[t][:], st,
                             xpt(k)[:, t * TBL:(t + 1) * TBL],
                             start=(i == 0), stop=(i == 7))
    for t in range(2):
        nc.scalar.copy(gsb[:, t * TBL:(t + 1) * TBL], g_ps[t][:])

    emit_buo(2, 0)

    # gt[:, st*64:(st+1)*64] = (G[:, st*128:(st+1)*128])^T via identity MMs
    gtsb = wp.tile([128, 512], F8D, tag="gt", name="gtsb")
    gt_ps = pC.tile([128, 512], DT, tag="pc", name="gt_ps")
    for st in range(8):
        nc.tensor.matmul(gt_ps[:, st * 64:(st + 1) * 64],
                         gsb[:, st * 128:(st + 1) * 128],
                         b2xsb[0:64, 1024:1088], start=True, stop=True)
    nc.scalar.copy(gtsb[:], gt_ps[:])

    emit_buo(3, 0)
    emit_buo(4, 0)

    # ---------------- j1 / j2 (with xtl-fed boundary tails) -----------------
    def emit_j(j, klo, khi, jps, tl):
        for i in range(klo, khi):
            k = KORD[i]
            st = w1t(k, j * 128, (j + 1) * 128)
            for t in range(2):
                nc.tensor.matmul(jps[t][:], st,
                                 xot(k)[:, t * TBL:(t + 1) * TBL],
                                 start=(i == 0), stop=(i == 7))
            nc.tensor.matmul(tl[:], st, xtlsb[:, k * 4:(k + 1) * 4],
                             start=(i == 0), stop=(i == 7))

    def evict_j(j, jps, tl):
        ka, kb = 2 * j - 1, 2 * j
        for t in range(2):
            nc.scalar.copy(
                pext[j][0:64, 4 + ka + t * TBL:4 + ka + (t + 1) * TBL],
                jps[t][0:64, :])
            nc.scalar.copy(
                pext[j][64:128, 4 + kb + t * TBL:4 + kb + (t + 1) * TBL],
                jps[t][64:128, :])
        nc.scalar.copy(pext[j][0:64, 4:4 + ka], tl[0:64, 4 - ka:4])
        nc.scalar.copy(pext[j][64:128, 4:4 + kb], tl[64:128, 4 - kb:4])

    j1ps = [pC.tile([128, TBL], DT, tag="pc", name=f"j1_ps{t}")
            for t in range(2)]
    tl1 = pA.tile([128, 4], DT, tag="pa", name="tl1")
    emit_j(1, 0, 4, j1ps, tl1)
    emit_buo(5, 0)

    # V[r, n] = sum_s G[r, s] A[n]^(1023-s), windowed by sorted-A tiles.
    # apv segments: [0:1024]=st7 all n; [1024:1280]=st6 n 768:1024;
    # [1280+st*128 ...]=st 0..5, n 896:1024.
    v_ps = [pC.tile([64, TBL], DT, tag="pc", name=f"v_ps{nh}")
            for nh in range(2)]
    nc.tensor.matmul(v_ps[0][:], gtsb[:, 448:512], apvsb[:, 0:512],
                     start=True, stop=True)
    nc.tensor.matmul(v_ps[1][:], gtsb[:, 448:512], apvsb[:, 512:1024],
                     start=True, stop=False)
    nc.tensor.matmul(v_ps[1][:, 256:512], gtsb[:, 384:448],
                     apvsb[:, 1024:1280], start=False, stop=False)
    for st in range(6):
        nc.tensor.matmul(v_ps[1][:, 384:512], gtsb[:, st * 64:(st + 1) * 64],
                         apvsb[:, 1280 + st * 128:1280 + (st + 1) * 128],
                         start=False, stop=(st == 5))

    # E = V * B2 elementwise (DVE — slots between scans 5 and 6);
    # D[n] = sum_r E[r, n] * (1/1024 ones-matmul)
    esb = wp.tile([64, 1024], MDT, tag="esb", name="esb")
    for nh in range(2):
        nc.vector.scalar_tensor_tensor(
            esb[:, nh * TBL:(nh + 1) * TBL], v_ps[nh][:], 1.0,
            b2xsb[0:64, nh * TBL:(nh + 1) * TBL], MULT, MULT)
    d_ps = pA.tile([128, 8], DT, tag="pa", name="d_ps")
    for n in range(8):
        nc.tensor.matmul(d_ps[:, n:n + 1], esb[:, n * 128:(n + 1) * 128],
                         b2xsb[0:64, 1088:1089], start=True, stop=True)

    emit_buo(6, 0)
    emit_j(1, 4, 8, j1ps, tl1)
    evict_j(1, j1ps, tl1)
    emit_buo(7, 0)

    # correction stationaries on the ACT engine: scorr = C1*D (chunk carry),
    # sc2 = C1*h_511 (block-1 zero-init carry).  Emitted after ALL b0 scans
    # so the hsb reads depend on the scan writes.
    scorr = wp.tile([128, 512], MDT, tag="scorr", name="scorr")
    sc2 = wp.tile([128, 512], MDT, tag="sc2", name="sc2")
    h511f = wp.tile([128, 8], DT, tag="h511f", name="h511f")
    dsb = wp.tile([128, 8], DT, tag="dsb", name="dsb")
    nc.scalar.copy(dsb[:], d_ps[:])
    for nt in range(8):
        nc.scalar.mul(scorr[:, nt * 64:(nt + 1) * 64],
                      c1sb[:, nt * 64:(nt + 1) * 64], dsb[:, nt:nt + 1])
    for nt in range(8):
        nc.scalar.copy(h511f[:, nt:nt + 1], hsb[nt][:, TBL - 1:TBL])
        nc.scalar.mul(sc2[:, nt * 64:(nt + 1) * 64],
                      c1sb[:, nt * 64:(nt + 1) * 64], h511f[:, nt:nt + 1])

    j2ps = [pC.tile([128, TBL], DT, tag="pc", name=f"j2_ps{t}")
            for t in range(2)]
    tl2 = pA.tile([128, 4], DT, tag="pa", name="tl2")
    emit_j(2, 0, 4, j2ps, tl2)
    emit_buo(0, 1)
    emit_j(2, 4, 8, j2ps, tl2)
    evict_j(2, j2ps, tl2)
    emit_buo(1, 1)

    # ---------------- CH1(tb) + windowed corrections, Y(tb) -----------------
    ysb = [[yp.tile([128, 4 * TBL], ODT, tag=f"y{tb}{g}", name=f"y{tb}{g}")
            for g in range(2)] for tb in range(2)]

    def corr_mms(cps, stat, last_stop):
        # windowed A^(t+1) correction: tile7 full 512, tile6 256, rest 128
        nc.tensor.matmul(cps[:], stat[:, 448:512], apcsb[:, 0:512],
                         start=False, stop=False)
        nc.tensor.matmul(cps[:, 0:256], stat[:, 384:448],
                         apcsb[:, 1024:1280], start=False, stop=False)
        for nt in range(6):
            nc.tensor.matmul(
                cps[:, 0:128], stat[:, nt * 64:(nt + 1) * 64],
                apcsb[:, 1280 + nt * 128:1280 + (nt + 1) * 128],
                start=False, stop=(last_stop and nt == 5))

    def emit_ch1(tb):
        cps = pC.tile([64, TBL], DT, tag="pc", name=f"c_ps{tb}")
        for n in range(8):
            nc.tensor.matmul(cps[:], c1sb[:, n * 64:(n + 1) * 64],
                             hsb[n][:, tb * TBL:(tb + 1) * TBL],
                             start=(n == 0), stop=False)
        if tb == 0:
            corr_mms(cps, scorr, True)
        else:
            # chunk carry at t 512:1024 decays below cutoff except tile 7
            nc.tensor.matmul(cps[:], scorr[:, 448:512], apcsb[:, 512:1024],
                             start=False, stop=False)
            corr_mms(cps, sc2, True)
        nc.scalar.copy(pext[0][0:64, 4 + tb * TBL:4 + (tb + 1) * TBL], cps[:])

    # Y: per o-tile accumulate m=1, m=2 early; the CH1-dependent m=0 last.
    yq = {}

    def y_mm12(tb, o):
        yps = pA.tile([128, TBL], DT, tag="pa", name=f"y_ps{o}_{tb}")
        yq[(tb, o)] = yps
        nc.tensor.matmul(yps[:], w2t(1, o),
                         pext[1][:, 4 + tb * TBL:4 + (tb + 1) * TBL],
                         start=True, stop=False)
        nc.tensor.matmul(yps[:], w2t(2, o),
                         pext[2][:, 4 + tb * TBL:4 + (tb + 1) * TBL],
                         start=False, stop=False)

    def y_mm0(tb, o):
        yps = yq.pop((tb, o))
        nc.tensor.matmul(yps[:], w2t(0, o),
                         pext[0][:, 4 + tb * TBL:4 + (tb + 1) * TBL],
                         start=False, stop=True)
        g, oo = divmod(o, 4)
        dst = ysb[tb][g][:, oo * TBL:(oo + 1) * TBL]
        if (tb == 0 and o < 6) or (tb == 1 and o % 2 == 0):
            nc.scalar.copy(dst, yps[:])
        else:
            nc.vector.tensor_copy(dst, yps[:])
        if oo == 3:
            eng = [nc.sync, nc.gpsimd, nc.gpsimd, nc.sync][tb * 2 + g]
            eng.dma_start(
                yt[:, tb * 4096 + g * 2048:tb * 4096 + (g + 1) * 2048],
                ysb[tb][g][:])

    emit_ch1(0)
    emit_buo(2, 1)

    # CH1(1): n-matmuls interleaved into the Y(0) pipeline (each waits only
    # its own b1 scan); the s(7,1)-dependent pieces + corrections come last.
    cps1 = pC.tile([64, TBL], DT, tag="pc", name="c_ps1")

    def ch1_1_n(n, start=False):
        nc.tensor.matmul(cps1[:], c1sb[:, n * 64:(n + 1) * 64],
                         hsb[n][:, TBL:TC], start=start, stop=False)

    ch1_1_n(0, start=True)
    y_mm12(0, 0)
    y_mm12(0, 1)
    y_mm12(0, 2)
    y_mm0(0, 0)
    emit_buo(3, 1)
    ch1_1_n(1)
    y_mm12(0, 3)
    y_mm0(0, 1)
    y_mm12(0, 4)
    y_mm0(0, 2)
    emit_buo(4, 1)
    ch1_1_n(2)
    y_mm12(0, 5)
    y_mm0(0, 3)
    y_mm12(0, 6)
    y_mm0(0, 4)
    emit_buo(5, 1)
    ch1_1_n(3)
    y_mm12(0, 7)
    y_mm0(0, 5)
    y_mm0(0, 6)
    emit_buo(6, 1)
    ch1_1_n(4)
    y_mm0(0, 7)
    y_mm12(1, 0)
    emit_buo(7, 1)
    ch1_1_n(5)
    y_mm12(1, 1)
    y_mm12(1, 2)
    ch1_1_n(6)
    ch1_1_n(7)
    nc.tensor.matmul(cps1[:], scorr[:, 448:512], apcsb[:, 512:1024],
                     start=False, stop=False)
    corr_mms(cps1, sc2, True)
    nc.scalar.copy(pext[0][0:64, 4 + TBL:4 + TC], cps1[:])
    y_mm0(1, 0)
    y_mm12(1, 3)
    y_mm0(1, 1)
    y_mm12(1, 4)
    y_mm0(1, 2)
    y_mm12(1, 5)
    y_mm0(1, 3)
    y_mm12(1, 6)
    y_mm0(1, 4)
    y_mm12(1, 7)
    y_mm0(1, 5)
    y_mm0(1, 6)
    y_mm0(1, 7)


def _build():
    nc = bacc.Bacc("TRN2", target_bir_lowering=False, debug=False,
                   num_devices=8)
    xo = nc.dram_tensor("xo", [128, 8192], MDT, kind="ExternalInput").ap()
    xp = nc.dram_tensor("xp", [128, 8192], F8D, kind="ExternalInput").ap()
    xtl = nc.dram_tensor("xtl", [128, 32], MDT, kind="ExternalInput").ap()
    w1 = nc.dram_tensor("w1", [128, 3072], MDT, kind="ExternalInput").ap()
    w1g8 = nc.dram_tensor("w1g8", [128, 512], F8D, kind="ExternalInput").ap()
    b2x = nc.dram_tensor("b2x", [128, B2W], MDT, kind="ExternalInput").ap()
    c1 = nc.dram_tensor("c1", [128, 512], MDT, kind="ExternalInput").ap()
    w2 = nc.dram_tensor("w2", [128, 3072], MDT, kind="ExternalInput").ap()
    apv = nc.dram_tensor("apv", [128, APW], F8D, kind="ExternalInput").ap()
    apc = nc.dram_tensor("apc", [128, APW], MDT, kind="ExternalInput").ap()
    avio = nc.dram_tensor("avio", [128, 16], DT, kind="ExternalInput").ap()
    yt = nc.dram_tensor("yt", [128, 8192], ODT, kind="ExternalOutput").ap()

    with tile.TileContext(nc) as tc, ExitStack() as ctx:
        _emit(ctx, tc, (xo, xp, xtl, w1, w1g8, b2x, c1, w2, apv, apc,
                        avio, yt))
    nc.compile()
    return nc


def _get_nc():
    global _CACHED_NC
    if _CACHED_NC is None:
        _CACHED_NC = _build()
    return _CACHED_NC


def _pack_kt(arr):
    """[1024, C] -> [128, 8*C] with blocks of 128 rows side by side."""
    C = arr.shape[1]
    return np.ascontiguousarray(
        arr.reshape(8, 128, C).transpose(1, 0, 2).reshape(128, 8 * C))


def kernel(inputs, h0, A, B1, B2, C1, C2, M1, M2):
    global LAST_RESULT
    X = np.asarray(inputs, dtype=F32)
    h0 = np.asarray(h0, dtype=F32)
    A = np.asarray(A, dtype=F32)

    # sort states by A ascending (weights-only permutation)
    perm = np.argsort(np.asarray(A, dtype=np.float64))
    As64 = np.asarray(A, dtype=np.float64)[perm]
    h0s = h0[perm]
    B2s = np.asarray(B2, dtype=F32)[:, perm]
    C1s = np.asarray(C1, dtype=F32)[perm, :]

    # sanity: windows hold for this A draw (program structure is fixed)
    assert As64[767] ** 128 < 1e-4, As64[767]
    assert As64[895] ** 256 < 1e-4, As64[895]

    W1 = np.concatenate(
        [np.asarray(B1, dtype=F32)]
        + [np.ascontiguousarray(np.asarray(M1, dtype=F32)[:, :, k].T)
           for k in range(KX)], axis=1)
    W2 = np.concatenate(
        [np.asarray(C2, dtype=F32)]
        + [np.ascontiguousarray(np.asarray(M2, dtype=F32)[:, :, k].T)
           for k in range(KX)], axis=0)
    w1kt = W1.astype(MNP).reshape(8, 128, 384)           # [k, p, c]
    w1p = np.concatenate([                               # [128, 1024 | 2048]
        np.ascontiguousarray(w1kt[:, :, 0:128].transpose(1, 0, 2)
                             .reshape(128, 1024)),
        np.ascontiguousarray(w1kt[:, :, 128:384].transpose(1, 0, 2)
                             .reshape(128, 2048))], axis=1)
    # fp8 B1 copy, pre-scaled x1024 into e4m3 range (G path only)
    b1s = (np.asarray(B1, dtype=F32) * 1024.0).astype(F8DNP)  # [1024, 64]
    w1g8p = np.ascontiguousarray(
        b1s.reshape(8, 128, 64).transpose(1, 0, 2).reshape(128, 512))
    w2p = np.ascontiguousarray(
        W2.reshape(3, 128, 1024).transpose(1, 0, 2)
        .reshape(128, 3072).astype(MNP))                 # [128, 3072]
    b2xm = np.zeros((128, B2W), F32)
    b2xm[0:64, 0:1024] = B2s
    b2xm[0:64, 1024:1088] = np.eye(64, dtype=F32)
    b2xm[64:128, 1024:1088] = np.eye(64, dtype=F32)
    b2xm[0:64, 1088] = 1.0 / 1024.0
    b2xp = np.ascontiguousarray(b2xm.astype(MNP))
    c1p = _pack_kt(C1s.astype(MNP))                      # [128, 512]

    # apv: V decay powers A^(1023-s), windowed; s = st*128 + p.
    lnAs = np.log(As64)
    p = np.arange(128, dtype=np.float64)
    apvm = np.zeros((128, APW), np.float64)
    apvm[:, 0:1024] = np.exp(np.outer(127.0 - p, lnAs))              # st7
    apvm[:, 1024:1280] = np.exp(np.outer(255.0 - p, lnAs[768:1024]))  # st6
    for st in range(6):
        apvm[:, 1280 + st * 128:1280 + (st + 1) * 128] = np.exp(
            np.outer(1023.0 - (st * 128 + p), lnAs[896:1024]))
    apvp = np.ascontiguousarray(apvm.astype(F32).astype(F8DNP))

    # apc: correction decay powers A^(t+1), windowed per sorted n-tile.
    t1 = np.arange(1, 1025, dtype=np.float64)
    apcm = np.zeros((128, APW), np.float64)
    apcm[:, 0:1024] = np.exp(np.outer(lnAs[896:1024], t1))           # tile7
    apcm[:, 1024:1280] = np.exp(np.outer(lnAs[768:896], t1[0:256]))  # tile6
    for nt in range(6):
        apcm[:, 1280 + nt * 128:1280 + (nt + 1) * 128] = np.exp(
            np.outer(lnAs[nt * 128:(nt + 1) * 128], t1[0:128]))
    apcp = np.ascontiguousarray(apcm.astype(F32).astype(MNP))

    ioff_h0 = h0s.astype(F32)                              # half 0: plain h0
    ioff_h1 = (As64 ** TC * h0s.astype(np.float64)).astype(F32)  # A^1024 h0

    Xbf = X.astype(MNP)
    X8 = X.astype(F8DNP)
    zeros_xp = np.zeros((128, 8192), F8DNP)
    zeros_xtl = np.zeros((128, 32), MNP)

    def pack_x(xarr, b, sl):
        return _pack_kt(np.ascontiguousarray(xarr[b, sl, :].T))

    in_maps = []
    for c in range(8):
        b, half = divmod(c, 2)
        xoc = pack_x(Xbf, b, slice(half * TC, (half + 1) * TC))
        if half == 0:
            xpc, xtlc, ioff = zeros_xp, zeros_xtl, ioff_h0
        else:
            xpc = pack_x(X8, b, slice(0, TC))
            xpbf = pack_x(Xbf, b, slice(0, TC))
            # xtl: last 4 time-cols of each k-tile of xp, [128, 8*4]
            xtlc = np.ascontiguousarray(
                xpbf.reshape(128, 8, 1024)[:, :, 1020:1024].reshape(128, 32))
            ioff = ioff_h1
        aviom = np.zeros((128, 16), F32)
        aviom[:, 0:8] = As64.astype(F32).reshape(8, 128).T
        aviom[:, 8:16] = ioff.reshape(8, 128).T
        in_maps.append({"xo": xoc, "xp": xpc, "xtl": xtlc, "w1": w1p,
                        "w1g8": w1g8p, "b2x": b2xp, "c1": c1p, "w2": w2p,
                        "apv": apvp, "apc": apcp, "avio": aviom})

    nc = _get_nc()
    trace = bool(int(os.environ.get("KERNEL_TRACE", "0")))
    LAST_RESULT = run_bass_kernel_spmd(nc, in_maps, core_ids=list(range(8)),
                                       trace=trace)
    Y = np.empty((B, T, OUT), F32)
    for c in range(8):
        b, half = divmod(c, 2)
        ytc = np.asarray(LAST_RESULT.results[c]["yt"], dtype=F32)
        # yt[p, tb*4096 + o*512 + t] -> Y_core[o*128+p, tb*512+t]
        yc = ytc.reshape(128, 2, 8, 512).transpose(2, 0, 1, 3).reshape(1024, 1024)
        Y[b, half * TC:(half + 1) * TC, :] = yc.T
    return Y


# revision 35
# speedup vs baseline: 1.1076x; 1.0020x over previous
"""Trainium2 Bass kernel for nn_LDS_LR: low-rank LDS + AR low-rank correction.

Math (per batch b):
    Bu   = X @ B1 @ B2                      # [T, N] rank-64 input projection
    h_t  = A * h_{t-1} + Bu_t               # diagonal recurrence, h_{-1} = h0
    lds  = H @ C1 @ C2                      # [T, O] rank-64 output projection
    proj = einsum('ti,rik->trk', X, M1)     # [T, R, KX]
    ar_t = sum_k M2[:,:,k] @ proj[t-k,:,k]  # AR with KX=5 taps
    Y    = lds + ar

Sharding: 8 cores = 4 batches x 2 sequence halves (1024 steps each).

v4 design notes (on top of v3's sorted-A windowed carries):
  * States host-permuted by |A| asc; windowed decay matmuls for the carry V
    and the CH1 corrections (~2k cols each instead of 8k).
  * Scan-then-correct at BOTH levels: block-1 scans start from zero, the
    missed A^(t'+1)*h_511 term is folded into CH1(1) as one more windowed
    matmul reusing the same apc slices.  All 16 scans are then independent:
    no serial 20us DVE chain; a few scans can offload to the Pool engine.
  * The carry-only inputs (xp, B1 copy, apv) travel as fp8e4m3 — the carry
    is a small additive term so 6% quantization there is ~0.3% on Y.  B1 is
    pre-scaled x1024 into fp8 range; the ones-reduction column carries the
    1/1024 compensation.
  * G matmul paired into PE column groups (rows 0:64 = tb0, 64:128 = tb1):
    half the column count and one eviction instead of two.
  * PE warm-up matmuls on a memset tile from t~0 keep the HAM clock-gate at
    2.4 GHz; input DMAs spread over the 3 hardware queues, k-loops consume
    in arrival order; xp scheduled mid-stream (carry chain needs it late).
"""

import contextlib
import ctypes
import os
import sys
import types

import numpy as np
from contextlib import ExitStack

import concourse.bass as bass
import concourse.tile as tile
from concourse import bacc, mybir
from concourse.bass_utils import run_bass_kernel_spmd


def _install_ntff_hook():
    """Provide antenv.axon_hooks.get_axon_ntff_profile_hook if the image
    lacks it, driving NTFF capture via the libaxon_pjrt C ABI directly."""
    try:
        from antenv.axon_hooks import get_axon_ntff_profile_hook  # noqa: F401
        return
    except ImportError:
        pass
    so_path = "/opt/axon/libaxon_pjrt.so"
    hook = None
    if os.path.exists(so_path):
        lib = ctypes.CDLL(so_path)
        if hasattr(lib, "axon_start_nrt_profile"):
            lib.axon_start_nrt_profile.argtypes = [
                ctypes.POINTER(ctypes.c_int64), ctypes.c_size_t]
            lib.axon_start_nrt_profile.restype = ctypes.c_int64
            lib.axon_stop_nrt_profile.argtypes = [ctypes.c_char_p]
            lib.axon_stop_nrt_profile.restype = ctypes.c_int64

            @contextlib.contextmanager
            def _hook(output_dir, device_ids):
                import jax
                jax.devices()
                if device_ids:
                    ids = (ctypes.c_int64 * len(device_ids))(*device_ids)
                    rc = lib.axon_start_nrt_profile(ids, len(device_ids))
                else:
                    rc = lib.axon_start_nrt_profile(None, 0)
                if rc != 0:
                    raise RuntimeError(f"axon_start_nrt_profile rc={rc}")
                try:
                    yield
                finally:
                    n = lib.axon_stop_nrt_profile(str(output_dir).encode())
                    print(f"ntff profile: {n} file(s) -> {output_dir}",
                          file=sys.stderr)

            hook = _hook
    mod = types.ModuleType("antenv.axon_hooks")
    mod.get_axon_ntff_profile_hook = lambda: hook
    mod.set_axon_ntff_profile_hook = lambda h: None
    sys.modules["antenv.axon_hooks"] = mod


_install_ntff_hook()

DT = mybir.dt.float32
MDT = mybir.dt.bfloat16
F8 = mybir.dt.float8e4
MNP = mybir.dt.np(MDT)
F8NP = mybir.dt.np(F8)
F32 = np.float32
ODT = MDT
ONP = mybir.dt.np(ODT)

B, T, D = 4, 2048, 1024
NST, R, KX, OUT = 1024, 64, 5, 1024
TC = 1024          # per-core chunk length
TBL = 512          # time block (one PSUM bank at fp32)

# decay windows per sorted n-tile (compile-time; states sorted by A asc)
APW = 2048         # apv/apc width: 1024 + 256 + 6*128
B2W = 1152         # b2x width: 1024 B2 + 64 ident + ones + pad

WARM_MM = int(os.environ.get("KERNEL_WARM_MM", "10"))
# scans whose Bu is staged PSUM->SBUF (ACT copy) so the DVE scan runs in the
# all-SBUF 2x mode; remaining scans read PSUM directly at 1x
STAGE_SCAN = int(os.environ.get("KERNEL_STAGE_SCAN", "0"))
USE_F8 = bool(int(os.environ.get("KERNEL_F8", "1")))      # fp8 carry path
# col-group-paired G matmul: faults trn2 hardware when the operands are fp8
# (bf16 pairing and unpaired fp8 both pass) — keep off
PAIR_G = bool(int(os.environ.get("KERNEL_PAIR_G", "0")))
F8D = F8 if USE_F8 else MDT   # dtype of the carry-path tensors
F8DNP = mybir.dt.np(F8D)

_CACHED_NC = None
LAST_RESULT = None  # BassKernelResults of the most recent run (for test.py)

MULT = mybir.AluOpType.mult
ADD = mybir.AluOpType.add

# xo k-tile consumption order ~ DMA arrival (q0,q2 sync / q1 gpsimd /
# q3 scalar-after-w1g)
KORD = [0, 4, 1, 5, 2, 6, 3, 7]


def _emit(ctx, tc, io):
    nc = tc.nc
    xo, xp, xtl, w1, w1g8, b2x, c1, w2, apv, apc, avio, yt = io

    wp = ctx.enter_context(tc.tile_pool(name="wp", bufs=1))
    xpool = ctx.enter_context(tc.tile_pool(name="xpool", bufs=1))
    hp = ctx.enter_context(tc.tile_pool(name="hp", bufs=1))
    pp = ctx.enter_context(tc.tile_pool(name="pp", bufs=1))
    yp = ctx.enter_context(tc.tile_pool(name="yp", bufs=1))
    pA = ctx.enter_context(tc.tile_pool(name="pA", bufs=3, space="PSUM"))
    pB = ctx.enter_context(tc.tile_pool(name="pB", bufs=3, space="PSUM"))
    pC = ctx.enter_context(tc.tile_pool(name="pC", bufs=2, space="PSUM"))

    # ---------------- warm-up: memset tile + dummy accumulating MMs ---------
    wtile = wp.tile([128, 640], MDT, tag="wtile", name="wtile")
    nc.gpsimd.memset(wtile[:], 0.0)
    wps = pC.tile([128, TBL], DT, tag="pc", name="wps")
    for i in range(WARM_MM):
        nc.tensor.matmul(wps[:], wtile[:, 512:640], wtile[:, 0:512],
                         start=(i == 0), stop=(i == WARM_MM - 1))

    # ---------------- input DMAs, 3 queues, deadline-ordered ----------------
    aviosb = wp.tile([128, 16], DT, tag="avio", name="aviosb")
    xots = [xpool.tile([128, 2048], MDT, tag=f"xoq{i}", name=f"xoq{i}")
            for i in range(4)]
    xpts = [xpool.tile([128, 2048], F8D, tag=f"xpq{i}", name=f"xpq{i}")
            for i in range(4)]
    c1sb = wp.tile([128, 512], MDT, tag="c1", name="c1sb")
    w1gsb = wp.tile([128, 1024], MDT, tag="w1g", name="w1gsb")
    w1rsb = wp.tile([128, 2048], MDT, tag="w1r", name="w1rsb")
    w1g8sb = wp.tile([128, 512], F8D, tag="w1g8", name="w1g8sb")
    xtlsb = wp.tile([128, 32], MDT, tag="xtl", name="xtlsb")
    b2xsb = wp.tile([128, B2W], MDT, tag="b2x", name="b2xsb")
    w2sb = wp.tile([128, 3072], MDT, tag="w2", name="w2sb")
    apvsb = wp.tile([128, APW], F8D, tag="apv", name="apvsb")
    apcsb = wp.tile([128, APW], MDT, tag="apc", name="apcsb")

    # sync queue: xo first, then xp quarter + corr weights
    nc.sync.dma_start(aviosb[:], avio[:])
    nc.sync.dma_start(xots[0][:], xo[:, 0:2048])
    nc.sync.dma_start(xots[1][:], xo[:, 2048:4096])
    nc.sync.dma_start(xpts[0][:], xp[:, 0:2048])
    nc.sync.dma_start(c1sb[:], c1[:])
    nc.sync.dma_start(apcsb[:], apc[:])
    # gpsimd queue: xo second half, then xp
    nc.gpsimd.dma_start(xots[2][:], xo[:, 4096:6144])
    nc.gpsimd.dma_start(xots[3][:], xo[:, 6144:8192])
    nc.gpsimd.dma_start(xpts[1][:], xp[:, 2048:4096])
    nc.gpsimd.dma_start(xpts[2][:], xp[:, 4096:6144])
    nc.gpsimd.dma_start(xpts[3][:], xp[:, 6144:8192])
    # scalar queue: all weights
    nc.scalar.dma_start(w1gsb[:], w1[:, 0:1024])
    nc.scalar.dma_start(w1g8sb[:], w1g8[:])
    nc.scalar.dma_start(b2xsb[:], b2x[:])
    nc.scalar.dma_start(w1rsb[:], w1[:, 1024:3072])
    nc.scalar.dma_start(xtlsb[:], xtl[:])
    nc.scalar.dma_start(apvsb[:], apv[:])
    nc.scalar.dma_start(w2sb[:], w2[:])

    def xot(k):
        return xots[k // 2][:, (k % 2) * 1024:(k % 2 + 1) * 1024]

    def xpt(k):
        return xpts[k // 2][:, (k % 2) * 1024:(k % 2 + 1) * 1024]

    def w1t(k, lo, hi):  # W1 k-tile column slice (w1g: cols 0:128, w1r: rest)
        if hi <= 128:
            return w1gsb[:, k * 128 + lo:k * 128 + hi]
        assert lo >= 128
        return w1rsb[:, k * 256 + lo - 128:k * 256 + hi - 128]

    def w2t(m, o):  # W2 stationary for (m-tile, o-tile) [128, 128]
        return w2sb[:, m * 1024 + o * 128:m * 1024 + (o + 1) * 128]

    def abv(n):  # A broadcast for scans, stride-0 partition view
        return aviosb[:, n:n + 1].broadcast_to((128, TBL))

    # ---------------- j0 = [B1|tap0]^T Xo -----------------------------------
    PW = 4 + TC + 4
    pext = [pp.tile([128, PW], MDT, tag=f"pext{j}", name=f"pext{j}")
            for j in range(3)]
    j0ps = [pA.tile([128, TBL], DT, tag="pa", name=f"j0_ps{t}")
            for t in range(2)]
    for i, k in enumerate(KORD):
        for t in range(2):
            nc.tensor.matmul(j0ps[t][:], w1t(k, 0, 128),
                             xot(k)[:, t * TBL:(t + 1) * TBL],
                             start=(i == 0), stop=(i == 7))
    for t in range(2):
        nc.scalar.copy(pext[0][:, 4 + t * TBL:4 + (t + 1) * TBL], j0ps[t][:])

    # ---------------- Bu + 16 independent scans -----------------------------
    # b0 scans start from the host-folded h0 offset; b1 scans start from 0 and
    # the missed A^(t'+1)*h_511 term lands in CH1(1) via the sc2 correction.
    hsb = [hp.tile([128, TC], MDT, tag=f"h{n}", name=f"h{n}") for n in range(8)]

    def emit_buo(n, tb):
        bu = pB.tile([128, TBL], DT, tag="pb", name=f"buo{n}_{tb}")
        nc.tensor.matmul(bu[:], b2xsb[0:64, n * 128:(n + 1) * 128],
                         pext[0][0:64, 4 + tb * TBL:4 + (tb + 1) * TBL],
                         start=True, stop=True)
        init = aviosb[:, 8 + n:9 + n] if tb == 0 else 0.0
        dst = hsb[n][:, tb * TBL:(tb + 1) * TBL]
        nc.vector.tensor_tensor_scan(dst, abv(n), bu[:], init, MULT, ADD)

    emit_buo(0, 0)
    emit_buo(1, 0)

    # ---------------- carry chain: G = B1^T Xp, G^T, V, E, D ----------------
    gsb = wp.tile([64, 1024], MDT, tag="gprev", name="gsb")
    g_ps = [pC.tile([64, TBL], DT, tag="pc", name=f"g_ps{t}")
            for t in range(2)]
    for i, k in enumerate(range(8)):
        st = w1g8sb[:, k * 64:(k + 1) * 64]
        for t in range(2):
            nc.tensor.matmul(g_ps[t][:], st,
                             xpt(k)[:, t * TBL:(t + 1) * TBL],
                             start=(i == 0), stop=(i == 7))
    for t in range(2):
        nc.scalar.copy(gsb[:, t * TBL:(t + 1) * TBL], g_ps[t][:])

    emit_buo(2, 0)

    # gt[:, st*64:(st+1)*64] = (G[:, st*128:(st+1)*128])^T via identity MMs
    gtsb = wp.tile([128, 512], F8D, tag="gt", name="gtsb")
    gt_ps = pC.tile([128, 512], DT, tag="pc", name="gt_ps")
    for st in range(8):
        nc.tensor.matmul(gt_ps[:, st * 64:(st + 1) * 64],
                         gsb[:, st * 128:(st + 1) * 128],
                         b2xsb[0:64, 1024:1088], start=True, stop=True)
    nc.scalar.copy(gtsb[:], gt_ps[:])

    emit_buo(3, 0)
    emit_buo(4, 0)

    # ---------------- j1 / j2 (with xtl-fed boundary tails) -----------------
    def emit_j(j, klo, khi, jps, tl):
        for i in range(klo, khi):
            k = KORD[i]
            st = w1t(k, j * 128, (j + 1) * 128)
            for t in range(2):
                nc.tensor.matmul(jps[t][:], st,
                                 xot(k)[:, t * TBL:(t + 1) * TBL],
                                 start=(i == 0), stop=(i == 7))
            nc.tensor.matmul(tl[:], st, xtlsb[:, k * 4:(k + 1) * 4],
                             start=(i == 0), stop=(i == 7))

    def evict_j(j, jps, tl):
        ka, kb = 2 * j - 1, 2 * j
        for t in range(2):
            nc.scalar.copy(
                pext[j][0:64, 4 + ka + t * TBL:4 + ka + (t + 1) * TBL],
                jps[t][0:64, :])
            nc.scalar.copy(
                pext[j][64:128, 4 + kb + t * TBL:4 + kb + (t + 1) * TBL],
                jps[t][64:128, :])
        nc.scalar.copy(pext[j][0:64, 4:4 + ka], tl[0:64, 4 - ka:4])
        nc.scalar.copy(pext[j][64:128, 4:4 + kb], tl[64:128, 4 - kb:4])

    j1ps = [pC.tile([128, TBL], DT, tag="pc", name=f"j1_ps{t}")
            for t in range(2)]
    tl1 = pA.tile([128, 4], DT, tag="pa", name="tl1")
    emit_j(1, 0, 4, j1ps, tl1)
    emit_buo(5, 0)

    # V[r, n] = sum_s G[r, s] A[n]^(1023-s), windowed by sorted-A tiles.
    # apv segments: [0:1024]=st7 all n; [1024:1280]=st6 n 768:1024;
    # [1280+st*128 ...]=st 0..5, n 896:1024.
    v_ps = [pC.tile([64, TBL], DT, tag="pc", name=f"v_ps{nh}")
            for nh in range(2)]
    nc.tensor.matmul(v_ps[0][:], gtsb[:, 448:512], apvsb[:, 0:512],
                     start=True, stop=True)
    nc.tensor.matmul(v_ps[1][:], gtsb[:, 448:512], apvsb[:, 512:1024],
                     start=True, stop=False)
    nc.tensor.matmul(v_ps[1][:, 256:512], gtsb[:, 384:448],
                     apvsb[:, 1024:1280], start=False, stop=False)
    for st in range(6):
        nc.tensor.matmul(v_ps[1][:, 384:512], gtsb[:, st * 64:(st + 1) * 64],
                         apvsb[:, 1280 + st * 128:1280 + (st + 1) * 128],
                         start=False, stop=(st == 5))

    # E = V * B2 elementwise (DVE — slots between scans 5 and 6);
    # D[n] = sum_r E[r, n] * (1/1024 ones-matmul)
    esb = wp.tile([64, 1024], MDT, tag="esb", name="esb")
    for nh in range(2):
        nc.vector.scalar_tensor_tensor(
            esb[:, nh * TBL:(nh + 1) * TBL], v_ps[nh][:], 1.0,
            b2xsb[0:64, nh * TBL:(nh + 1) * TBL], MULT, MULT)
    d_ps = pA.tile([128, 8], DT, tag="pa", name="d_ps")
    for n in range(8):
        nc.tensor.matmul(d_ps[:, n:n + 1], esb[:, n * 128:(n + 1) * 128],
                         b2xsb[0:64, 1088:1089], start=True, stop=True)

    emit_buo(6, 0)
    emit_j(1, 4, 8, j1ps, tl1)
    evict_j(1, j1ps, tl1)
    emit_buo(7, 0)

    # correction stationaries on the ACT engine: scorr = C1*D (chunk carry),
    # sc2 = C1*h_511 (block-1 zero-init carry).  Emitted after ALL b0 scans
    # so the hsb reads depend on the scan writes.
    scorr = wp.tile([128, 512], MDT, tag="scorr", name="scorr")
    sc2 = wp.tile([128, 512], MDT, tag="sc2", name="sc2")
    h511f = wp.tile([128, 8], DT, tag="h511f", name="h511f")
    dsb = wp.tile([128, 8], DT, tag="dsb", name="dsb")
    nc.scalar.copy(dsb[:], d_ps[:])
    for nt in range(8):
        nc.scalar.mul(scorr[:, nt * 64:(nt + 1) * 64],
                      c1sb[:, nt * 64:(nt + 1) * 64], dsb[:, nt:nt + 1])
    for nt in range(8):
        nc.scalar.copy(h511f[:, nt:nt + 1], hsb[nt][:, TBL - 1:TBL])
        nc.scalar.mul(sc2[:, nt * 64:(nt + 1) * 64],
                      c1sb[:, nt * 64:(nt + 1) * 64], h511f[:, nt:nt + 1])

    j2ps = [pC.tile([128, TBL], DT, tag="pc", name=f"j2_ps{t}")
            for t in range(2)]
    tl2 = pA.tile([128, 4], DT, tag="pa", name="tl2")
    emit_j(2, 0, 4, j2ps, tl2)
    emit_buo(0, 1)
    emit_j(2, 4, 8, j2ps, tl2)
    evict_j(2, j2ps, tl2)
    emit_buo(1, 1)

    # ---------------- CH1(tb) + windowed corrections, Y(tb) -----------------
    ysb = [[yp.tile([128, 4 * TBL], ODT, tag=f"y{tb}{g}", name=f"y{tb}{g}")
            for g in range(2)] for tb in range(2)]

    def corr_mms(cps, stat, last_stop):
        # windowed A^(t+1) correction: tile7 full 512, tile6 256, rest 128
        nc.tensor.matmul(cps[:], stat[:, 448:512], apcsb[:, 0:512],
                         start=False, stop=False)
        nc.tensor.matmul(cps[:, 0:256], stat[:, 384:448],
                         apcsb[:, 1024:1280], start=False, stop=False)
        for nt in range(6):
            nc.tensor.matmul(
                cps[:, 0:128], stat[:, nt * 64:(nt + 1) * 64],
                apcsb[:, 1280 + nt * 128:1280 + (nt + 1) * 128],
                start=False, stop=(last_stop and nt == 5))

    def emit_ch1(tb):
        cps = pC.tile([64, TBL], DT, tag="pc", name=f"c_ps{tb}")
        for n in range(8):
            nc.tensor.matmul(cps[:], c1sb[:, n * 64:(n + 1) * 64],
                             hsb[n][:, tb * TBL:(tb + 1) * TBL],
                             start=(n == 0), stop=False)
        if tb == 0:
            corr_mms(cps, scorr, True)
        else:
            # chunk carry at t 512:1024 decays below cutoff except tile 7
            nc.tensor.matmul(cps[:], scorr[:, 448:512], apcsb[:, 512:1024],
                             start=False, stop=False)
            corr_mms(cps, sc2, True)
        nc.scalar.copy(pext[0][0:64, 4 + tb * TBL:4 + (tb + 1) * TBL], cps[:])

    # Y: per o-tile accumulate m=1, m=2 early; the CH1-dependent m=0 last.
    yq = {}

    def y_mm12(tb, o):
        yps = pA.tile([128, TBL], DT, tag="pa", name=f"y_ps{o}_{tb}")
        yq[(tb, o)] = yps
        nc.tensor.matmul(yps[:], w2t(1, o),
                         pext[1][:, 4 + tb * TBL:4 + (tb + 1) * TBL],
                         start=True, stop=False)
        nc.tensor.matmul(yps[:], w2t(2, o),
                         pext[2][:, 4 + tb * TBL:4 + (tb + 1) * TBL],
                         start=False, stop=False)

    def y_mm0(tb, o):
        yps = yq.pop((tb, o))
        nc.tensor.matmul(yps[:], w2t(0, o),
                         pext[0][:, 4 + tb * TBL:4 + (tb + 1) * TBL],
                         start=False, stop=True)
        g, oo = divmod(o, 4)
        dst = ysb[tb][g][:, oo * TBL:(oo + 1) * TBL]
        if (tb == 0 and o < 6) or (tb == 1 and o % 2 == 0):
            nc.scalar.copy(dst, yps[:])
        else:
            nc.vector.tensor_copy(dst, yps[:])
        if oo == 3:
            eng = [nc.sync, nc.gpsimd, nc.scalar, nc.sync][tb * 2 + g]
            eng.dma_start(
                yt[:, tb * 4096 + g * 2048:tb * 4096 + (g + 1) * 2048],
                ysb[tb][g][:])

    emit_ch1(0)
    emit_buo(2, 1)

    # CH1(1): n-matmuls interleaved into the Y(0) pipeline (each waits only
    # its own b1 scan); the s(7,1)-dependent pieces + corrections come last.
    cps1 = pC.tile([64, TBL], DT, tag="pc", name="c_ps1")

    def ch1_1_n(n, start=False):
        nc.tensor.matmul(cps1[:], c1sb[:, n * 64:(n + 1) * 64],
                         hsb[n][:, TBL:TC], start=start, stop=False)

    ch1_1_n(0, start=True)
    y_mm12(0, 0)
    y_mm12(0, 1)
    y_mm12(0, 2)
    y_mm0(0, 0)
    emit_buo(3, 1)
    ch1_1_n(1)
    y_mm12(0, 3)
    y_mm0(0, 1)
    y_mm12(0, 4)
    y_mm0(0, 2)
    emit_buo(4, 1)
    ch1_1_n(2)
    y_mm12(0, 5)
    emit_buo(5, 1)
    y_mm0(0, 3)
    y_mm12(0, 6)
    y_mm0(0, 4)
    emit_buo(6, 1)
    ch1_1_n(3)
    y_mm12(0, 7)
    emit_buo(7, 1)
    y_mm0(0, 5)
    y_mm0(0, 6)
    ch1_1_n(4)
    y_mm0(0, 7)
    y_mm12(1, 0)
    ch1_1_n(5)
    y_mm12(1, 1)
    y_mm12(1, 2)
    ch1_1_n(6)
    ch1_1_n(7)
    nc.tensor.matmul(cps1[:], scorr[:, 448:512], apcsb[:, 512:1024],
                     start=False, stop=False)
    corr_mms(cps1, sc2, True)
    nc.scalar.copy(pext[0][0:64, 4 + TBL:4 + TC], cps1[:])
    y_mm0(1, 0)
    y_mm12(1, 3)
    y_mm0(1, 1)
    y_mm12(1, 4)
    y_mm0(1, 2)
    y_mm12(1, 5)
    y_mm0(1, 3)
    y_mm12(1, 6)
    y_mm0(1, 4)
    y_mm12(1, 7)
    y_mm0(1, 5)
    y_mm0(1, 6)
    y_mm0(1, 7)


def _build():
    nc = bacc.Bacc("TRN2", target_bir_lowering=False, debug=False,
                   num_devices=8)
    xo = nc.dram_tensor("xo", [128, 8192], MDT, kind="ExternalInput").ap()
    xp = nc.dram_tensor("xp", [128, 8192], F8D, kind="ExternalInput").ap()
    xtl = nc.dram_tensor("xtl", [128, 32], MDT, kind="ExternalInput").ap()
    w1 = nc.dram_tensor("w1", [128, 3072], MDT, kind="ExternalInput").ap()
    w1g8 = nc.dram_tensor("w1g8", [128, 512], F8D, kind="ExternalInput").ap()
    b2x = nc.dram_tensor("b2x", [128, B2W], MDT, kind="ExternalInput").ap()
    c1 = nc.dram_tensor("c1", [128, 512], MDT, kind="ExternalInput").ap()
    w2 = nc.dram_tensor("w2", [128, 3072], MDT, kind="ExternalInput").ap()
    apv = nc.dram_tensor("apv", [128, APW], F8D, kind="ExternalInput").ap()
    apc = nc.dram_tensor("apc", [128, APW], MDT, kind="ExternalInput").ap()
    avio = nc.dram_tensor("avio", [128, 16], DT, kind="ExternalInput").ap()
    yt = nc.dram_tensor("yt", [128, 8192], ODT, kind="ExternalOutput").ap()

    with tile.TileContext(nc) as tc, ExitStack() as ctx:
        _emit(ctx, tc, (xo, xp, xtl, w1, w1g8, b2x, c1, w2, apv, apc,
                        avio, yt))
    nc.compile()
    return nc


def _get_nc():
    global _CACHED_NC
    if _CACHED_NC is None:
        _CACHED_NC = _build()
    return _CACHED_NC


def _pack_kt(arr):
    """[1024, C] -> [128, 8*C] with blocks of 128 rows side by side."""
    C = arr.shape[1]
    return np.ascontiguousarray(
        arr.reshape(8, 128, C).transpose(1, 0, 2).reshape(128, 8 * C))


def kernel(inputs, h0, A, B1, B2, C1, C2, M1, M2):
    global LAST_RESULT
    X = np.asarray(inputs, dtype=F32)
    h0 = np.asarray(h0, dtype=F32)
    A = np.asarray(A, dtype=F32)

    # sort states by A ascending (weights-only permutation)
    perm = np.argsort(np.asarray(A, dtype=np.float64))
    As64 = np.asarray(A, dtype=np.float64)[perm]
    h0s = h0[perm]
    B2s = np.asarray(B2, dtype=F32)[:, perm]
    C1s = np.asarray(C1, dtype=F32)[perm, :]

    # sanity: windows hold for this A draw (program structure is fixed)
    assert As64[767] ** 128 < 1e-4, As64[767]
    assert As64[895] ** 256 < 1e-4, As64[895]

    W1 = np.concatenate(
        [np.asarray(B1, dtype=F32)]
        + [np.ascontiguousarray(np.asarray(M1, dtype=F32)[:, :, k].T)
           for k in range(KX)], axis=1)
    W2 = np.concatenate(
        [np.asarray(C2, dtype=F32)]
        + [np.ascontiguousarray(np.asarray(M2, dtype=F32)[:, :, k].T)
           for k in range(KX)], axis=0)
    w1kt = W1.astype(MNP).reshape(8, 128, 384)           # [k, p, c]
    w1p = np.concatenate([                               # [128, 1024 | 2048]
        np.ascontiguousarray(w1kt[:, :, 0:128].transpose(1, 0, 2)
                             .reshape(128, 1024)),
        np.ascontiguousarray(w1kt[:, :, 128:384].transpose(1, 0, 2)
                             .reshape(128, 2048))], axis=1)
    # fp8 B1 copy, pre-scaled x1024 into e4m3 range (G path only)
    b1s = (np.asarray(B1, dtype=F32) * 1024.0).astype(F8DNP)  # [1024, 64]
    w1g8p = np.ascontiguousarray(
        b1s.reshape(8, 128, 64).transpose(1, 0, 2).reshape(128, 512))
    w2p = np.ascontiguousarray(
        W2.reshape(3, 128, 1024).transpose(1, 0, 2)
        .reshape(128, 3072).astype(MNP))                 # [128, 3072]
    b2xm = np.zeros((128, B2W), F32)
    b2xm[0:64, 0:1024] = B2s
    b2xm[0:64, 1024:1088] = np.eye(64, dtype=F32)
    b2xm[64:128, 1024:1088] = np.eye(64, dtype=F32)
    b2xm[0:64, 1088] = 1.0 / 1024.0
    b2xp = np.ascontiguousarray(b2xm.astype(MNP))
    c1p = _pack_kt(C1s.astype(MNP))                      # [128, 512]

    # apv: V decay powers A^(1023-s), windowed; s = st*128 + p.
    lnAs = np.log(As64)
    p = np.arange(128, dtype=np.float64)
    apvm = np.zeros((128, APW), np.float64)
    apvm[:, 0:1024] = np.exp(np.outer(127.0 - p, lnAs))              # st7
    apvm[:, 1024:1280] = np.exp(np.outer(255.0 - p, lnAs[768:1024]))  # st6
    for st in range(6):
        apvm[:, 1280 + st * 128:1280 + (st + 1) * 128] = np.exp(
            np.outer(1023.0 - (st * 128 + p), lnAs[896:1024]))
    apvp = np.ascontiguousarray(apvm.astype(F32).astype(F8DNP))

    # apc: correction decay powers A^(t+1), windowed per sorted n-tile.
    t1 = np.arange(1, 1025, dtype=np.float64)
    apcm = np.zeros((128, APW), np.float64)
    apcm[:, 0:1024] = np.exp(np.outer(lnAs[896:1024], t1))           # tile7
    apcm[:, 1024:1280] = np.exp(np.outer(lnAs[768:896], t1[0:256]))  # tile6
    for nt in range(6):
        apcm[:, 1280 + nt * 128:1280 + (nt + 1) * 128] = np.exp(
            np.outer(lnAs[nt * 128:(nt + 1) * 128], t1[0:128]))
    apcp = np.ascontiguousarray(apcm.astype(F32).astype(MNP))

    ioff_h0 = h0s.astype(F32)                              # half 0: plain h0
    ioff_h1 = (As64 ** TC * h0s.astype(np.float64)).astype(F32)  # A^1024 h0

    Xbf = X.astype(MNP)
    X8 = X.astype(F8DNP)
    zeros_xp = np.zeros((128, 8192), F8DNP)
    zeros_xtl = np.zeros((128, 32), MNP)

    def pack_x(xarr, b, sl):
        return _pack_kt(np.ascontiguousarray(xarr[b, sl, :].T))

    in_maps = []
    for c in range(8):
        b, half = divmod(c, 2)
        xoc = pack_x(Xbf, b, slice(half * TC, (half + 1) * TC))
        if half == 0:
            xpc, xtlc, ioff = zeros_xp, zeros_xtl, ioff_h0
        else:
            xpc = pack_x(X8, b, slice(0, TC))
            xpbf = pack_x(Xbf, b, slice(0, TC))
            # xtl: last 4 time-cols of each k-tile of xp, [128, 8*4]
            xtlc = np.ascontiguousarray(
                xpbf.reshape(128, 8, 1024)[:, :, 1020:1024].reshape(128, 32))
            ioff = ioff_h1
        aviom = np.zeros((128, 16), F32)
        aviom[:, 0:8] = As64.astype(F32).reshape(8, 128).T
        aviom[:, 8:16] = ioff.reshape(8, 128).T
        in_maps.append({"xo": xoc, "xp": xpc, "xtl": xtlc, "w1": w1p,
                        "w1g8": w1g8p, "b2x": b2xp, "c1": c1p, "w2": w2p,
                        "apv": apvp, "apc": apcp, "avio": aviom})

    nc = _get_nc()
    trace = bool(int(os.environ.get("KERNEL_TRACE", "0")))
    LAST_RESULT = run_bass_kernel_spmd(nc, in_maps, core_ids=list(range(8)),
                                       trace=trace)
    Y = np.empty((B, T, OUT), F32)
    for c in range(8):
        b, half = divmod(c, 2)
        ytc = np.asarray(LAST_RESULT.results[c]["yt"], dtype=F32)
        # yt[p, tb*4096 + o*512 + t] -> Y_core[o*128+p, tb*512+t]
        yc = ytc.reshape(128, 2, 8, 512).transpose(2, 0, 1, 3).reshape(1024, 1024)
        Y[b, half * TC:(half + 1) * TC, :] = yc.T
    return Y
